# revision 1
# baseline (speedup 1.0000x reference)
"""GroupNorm + per-frame spatial attention block on 8 TRN2 NeuronCores.

Problem shape: x (1, 512, 4, 64, 64) f32.
  y   = GroupNorm32(x) (stats over (c/32, t, h, w) -> global over all frames)
  tok = y as (t, hw=4096, c=512)
  q,k,v = tok @ w{q,k,v}.T + b ; per-frame softmax(q k^T / sqrt(c)) v
  out = attn @ wp.T + bp ; return x + out

Sharding: core i handles frame f=i//2, query-half h=i%2 (2048 queries).
Each core redundantly computes K/V for its whole frame (cheaper than an
intra-pair all-gather).

Two launches (a fleet-wide collective barrier costs ~65us of latency, so
the tiny GroupNorm stats reduction is done as its own collective-free
kernel; the host combines the 8x[128,8] partial sums while "gathering"):
  kernel 1: per-core partial sum/sumsq over its disjoint half-frame.
  host:     combine partials -> per-channel scale/bias (512 numbers).
  kernel 2: normalize + qkv + attention + proj + residual.

Math simplifications used (exact, not approximations):
  - bk drops out of softmax (adds a per-query constant to scores).
  - bv passes through attention unchanged (softmax weights sum to 1), so
    it is folded into the proj bias on the host: bp_eff = bp + wp @ bv.
  - softmax computed without max-subtraction: scores ~ N(0,1) after the
    1/sqrt(c) scaling, exp() is safe in f32.

Device layouts (per core):
  xf   [512, 4096] f32 : frame, columns rolled so the local half is first
  w*T  [512, 512] bf16 : transposed weights [c_in, c_out] (contraction on
                         partitions)
  Scores are computed transposed, sT[kt,qt] = k_cm^T q_cm, so the exp'd
  probabilities feed the PV matmul (channel-major out) with zero on-chip
  transposes.  The softmax denominator is accumulated on the vector
  engine (f32), partition-reduced with a ones-matmul, reciprocal'd, and
  rank-1-broadcast on the PE; since proj is linear, normalization is
  applied after the proj matmul so the PV psum banks free up immediately.

Measured on 8xTRN2 (NTFF profile): ~27.5us (stats) + ~392us (main)
~= 420us total; main kernel TensorE-active ~331us (~80% occupancy,
~90% of bf16 stream peak while active).
"""

import numpy as np
import ml_dtypes

import concourse.bass as bass
import concourse.bacc as bacc
import concourse.tile as tile
from concourse import mybir
from concourse.bass_utils import run_bass_kernel_spmd

C = 512
T = 4
HW = 64 * 64          # tokens per frame
HALF = HW // 2        # local queries per core
G = 32                # groups
N_CORES = 8
EPS = 1e-6
NG_ELEMS = (C // G) * T * HW   # elements per group in the full tensor
CB = C // 128         # 4 channel blocks
QG = HALF // 512      # 4 query groups of 512
NKT = HW // 128       # 32 key chunks of 128
SCALE = float(C) ** -0.5

BF16 = mybir.dt.bfloat16
F32 = mybir.dt.float32
AX = mybir.AxisListType
AF = mybir.ActivationFunctionType
OP = mybir.AluOpType

_CACHE = {}


# ---------------------------------------------------------------- kernel 1
def _build_stats():
    nc = bacc.Bacc("TRN2", target_bir_lowering=False, debug=False,
                   num_devices=N_CORES)
    xh = nc.declare_dram_parameter("xh", [C, HALF], F32, isOutput=False)
    pstats = nc.declare_dram_parameter("pstats", [128, 2 * CB], F32,
                                       isOutput=True)
    with tile.TileContext(nc) as tc:
        with tc.tile_pool(name="xt", bufs=CB) as xt_pool, \
             tc.tile_pool(name="scr", bufs=2) as scr_pool, \
             tc.tile_pool(name="st", bufs=1) as st_pool:
            # sums on DVE, sums-of-squares on ACT: the two run in parallel
            stats_sb = st_pool.tile([128, 2 * CB], F32, name="stats")
            stats2_sb = st_pool.tile([128, CB], F32, name="stats2")
            for j in range(CB):
                xt = xt_pool.tile([128, HALF], F32, tag="xt", name="xt")
                eng = nc.sync if j % 2 == 0 else nc.scalar
                eng.dma_start(xt[:, :], xh[j * 128:(j + 1) * 128, :])
                nc.vector.reduce_sum(stats_sb[:, j:j + 1], xt[:, :], axis=AX.X)
                scr = scr_pool.tile([128, HALF], F32, tag="scr", name="scr")
                nc.scalar.activation(scr[:, :], xt[:, :], AF.Square,
                                     accum_out=stats2_sb[:, j:j + 1])
            nc.vector.tensor_copy(stats_sb[:, CB:2 * CB], stats2_sb[:, :])
            nc.sync.dma_start(pstats[:, :], stats_sb[:, :])
    nc.finalize()
    return nc


# ---------------------------------------------------------------- kernel 2
def _body(tc, P):
    from contextlib import ExitStack

    nc = tc.nc
    with ExitStack() as ctx:
        consts = ctx.enter_context(tc.tile_pool(name="consts", bufs=1))

        def load_const(name, shape, dtype, src, engine=None):
            t_ = consts.tile(shape, dtype, name=name)
            (engine or nc.scalar).dma_start(t_[:, :], src)
            return t_

        # The xn chain is the critical path: scale/bias ride the scalar
        # HWDGE ring, the 8MB xf load rides the sync ring in half-tiles
        # (the two rings share the 16 SDMA engines, so splitting xf across
        # them measured neutral), and the weights follow on the scalar ring.
        scl_sb = load_const("scl", [128, CB], F32, P["scl2d"][:, :])
        bia_sb = load_const("bia", [128, CB], F32, P["bia2d"][:, :])

        xn_pool = ctx.enter_context(tc.tile_pool(name="xn", bufs=CB))
        xn_sb = [xn_pool.tile([128, HW], BF16, tag="xn", name="xn") for _ in range(CB)]
        with tc.tile_pool(name="xf", bufs=3) as xf_pool:
            for half in range(2):
                cs = slice(half * (HW // 2), (half + 1) * (HW // 2))
                for j in range(CB):
                    xt = xf_pool.tile([128, HW // 2], F32, tag="xf", name="xf")
                    nc.sync.dma_start(xt[:, :], P["xf"][j * 128:(j + 1) * 128, cs])
                    nc.vector.tensor_scalar(
                        out=xn_sb[j][:, cs], in0=xt[:, :],
                        scalar1=scl_sb[:, j:j + 1], scalar2=bia_sb[:, j:j + 1],
                        op0=OP.mult, op1=OP.add)

        wq_sb = [load_const(f"wq{j}", [128, C], BF16, P["wqT"][j * 128:(j + 1) * 128, :]) for j in range(CB)]
        wk_sb = [load_const(f"wk{j}", [128, C], BF16, P["wkT"][j * 128:(j + 1) * 128, :]) for j in range(CB)]
        wv_sb = [load_const(f"wv{j}", [128, C], BF16, P["wvT"][j * 128:(j + 1) * 128, :]) for j in range(CB)]
        wp_sb = [load_const(f"wp{j}", [128, C], BF16, P["wpT"][j * 128:(j + 1) * 128, :]) for j in range(CB)]
        bq_sb = load_const("bq", [128, CB], F32, P["bq2d"][:, :])
        bpe_sb = load_const("bpe", [128, CB], F32, P["bpe2d"][:, :])
        onesf_sb = consts.tile([128, 1], F32, name="onesf")
        nc.vector.memset(onesf_sb[:, :], 1.0)
        onesrow_sb = consts.tile([1, 128], F32, name="onesrow")
        nc.vector.memset(onesrow_sb[:, :], 1.0)

        q_pool = ctx.enter_context(tc.tile_pool(name="q", bufs=CB))
        q_sb = [q_pool.tile([128, HALF], BF16, tag="q", name="q") for _ in range(CB)]
        k_pool = ctx.enter_context(tc.tile_pool(name="k", bufs=CB))
        k_sb = [k_pool.tile([128, HW], BF16, tag="k", name="k") for _ in range(CB)]
        v_pool = ctx.enter_context(tc.tile_pool(name="v", bufs=NKT))
        v_sb = [v_pool.tile([128, C], BF16, tag="v", name="v") for _ in range(NKT)]

        # psum pools: 4 + 3 + 1 = 8 banks
        ps_mm = ctx.enter_context(tc.tile_pool(name="ps_mm", bufs=4, space="PSUM"))
        ps_st = ctx.enter_context(tc.tile_pool(name="ps_st", bufs=3, space="PSUM"))
        ps_dn = ctx.enter_context(tc.tile_pool(name="ps_dn", bufs=1, space="PSUM"))

        p_pool = ctx.enter_context(tc.tile_pool(name="p", bufs=3))
        acc_pool = ctx.enter_context(tc.tile_pool(name="acc", bufs=2))
        dnr_pool = ctx.enter_context(tc.tile_pool(name="dnr", bufs=2))
        bc_pool = ctx.enter_context(tc.tile_pool(name="bc", bufs=2))
        atB_pool = ctx.enter_context(tc.tile_pool(name="atB", bufs=8))
        xr_pool = ctx.enter_context(tc.tile_pool(name="xr", bufs=3))
        ob_pool = ctx.enter_context(tc.tile_pool(name="ob", bufs=3))

        # ---------------- phase 1: q, k (channel-major), v (token-major) ----
        # Emit ALL work that only touches the first half of the frame
        # (q entirely + k/v first half) before anything needing the second
        # half: the second half's DMA+normalize is still in flight while
        # the PE chews through ~40us of first-half matmuls.
        def qk_group(w_sb, out_sb, j, t_, bias=None):
            ps = ps_mm.tile([128, 512], F32, tag="mm", name="mm")
            for ci in range(CB):
                nc.tensor.matmul(ps[:, :],
                                 lhsT=w_sb[ci][:, j * 128:(j + 1) * 128],
                                 rhs=xn_sb[ci][:, t_ * 512:(t_ + 1) * 512],
                                 start=(ci == 0), stop=(ci == CB - 1))
            dst = out_sb[j][:, t_ * 512:(t_ + 1) * 512]
            if bias is not None:
                nc.scalar.activation(dst, ps[:, :], AF.Identity, bias=bias)
            else:
                nc.scalar.copy(dst, ps[:, :])

        def v_group(m):
            ps = ps_mm.tile([128, 512], F32, tag="mm", name="mm")
            for ci in range(CB):
                nc.tensor.matmul(ps[:, :],
                                 lhsT=xn_sb[ci][:, m * 128:(m + 1) * 128],
                                 rhs=wv_sb[ci][:, :],
                                 start=(ci == 0), stop=(ci == CB - 1))
            nc.vector.tensor_copy(v_sb[m][:, :], ps[:, :])

        for j in range(CB):          # q covers exactly the first half
            for t_ in range(QG):
                qk_group(wq_sb, q_sb, j, t_, bias=bq_sb[:, j:j + 1])
        for j in range(CB):          # k, first half
            for t_ in range(4):
                qk_group(wk_sb, k_sb, j, t_)
        for m in range(NKT // 2):    # v, first half
            v_group(m)
        for j in range(CB):          # k, second half
            for t_ in range(4, 8):
                qk_group(wk_sb, k_sb, j, t_)
        for m in range(NKT // 2, NKT):
            v_group(m)

        # ---------------- phase 2: attention + proj per query group --------
        # proj of group g is emitted at the START of group g+1: its matmuls
        # are ready instantly (own psum pool, inputs done) and fill the PE
        # window where the next score matmuls wait on the denominator lag.
        def emit_proj(atB_sb, bc, q0):
            for cb in range(CB):
                pp = ps_dn.tile([128, 512], F32, tag="dn", name="pp")
                for j in range(CB):
                    nc.tensor.matmul(pp[:, :],
                                     lhsT=wp_sb[j][:, cb * 128:(cb + 1) * 128],
                                     rhs=atB_sb[j][:, :],
                                     start=(j == 0), stop=(j == CB - 1))
                xr = xr_pool.tile([128, 512], F32, tag="xr", name="xr")
                nc.scalar.dma_start(xr[:, :], P["xf"][cb * 128:(cb + 1) * 128, q0:q0 + 512])
                t1 = ob_pool.tile([128, 512], F32, tag="t1", name="t1")
                nc.vector.tensor_mul(t1[:, :], pp[:, :], bc[:, :])
                ob = ob_pool.tile([128, 512], F32, tag="ob", name="ob")
                nc.vector.scalar_tensor_tensor(ob[:, :], in0=t1[:, :],
                                               scalar=bpe_sb[:, cb:cb + 1],
                                               in1=xr[:, :],
                                               op0=OP.add, op1=OP.add)
                nc.sync.dma_start(P["out"][cb * 128:(cb + 1) * 128, q0:q0 + 512], ob[:, :])

        deferred = None
        for qg in range(QG):
            q0 = qg * 512
            pv = [ps_mm.tile([128, 512], F32, tag="mm", name="mm") for _ in range(CB)]
            if deferred is not None:
                emit_proj(*deferred)
                deferred = None
            acc = acc_pool.tile([128, 512], F32, tag="acc", name="acc")
            for m in range(NKT):
                st = ps_st.tile([128, 512], F32, tag="st", name="st")
                for j in range(CB):
                    nc.tensor.matmul(st[:, :],
                                     lhsT=k_sb[j][:, m * 128:(m + 1) * 128],
                                     rhs=q_sb[j][:, q0:q0 + 512],
                                     start=(j == 0), stop=(j == CB - 1))
                p = p_pool.tile([128, 512], BF16, tag="p", name="p")
                nc.scalar.activation(p[:, :], st[:, :], AF.Exp, scale=SCALE)
                if m == 0:
                    nc.vector.tensor_copy(acc[:, :], p[:, :])
                else:
                    nc.vector.tensor_add(acc[:, :], acc[:, :], p[:, :])
                for cb in range(CB):
                    # attention output channel-major: out[co, qt] += v^T p
                    nc.tensor.matmul(pv[cb][:, :],
                                     lhsT=v_sb[m][:, cb * 128:(cb + 1) * 128],
                                     rhs=p[:, :],
                                     start=(m == 0), stop=(m == NKT - 1))
            # copy UNNORMALIZED attention out of PSUM right away (frees the
            # pv banks for the next query group); the softmax denominator is
            # applied after the (linear) projection instead.
            atB_sb = []
            for cb in range(CB):
                atB = atB_pool.tile([128, 512], BF16, tag="atB", name="atB")
                nc.scalar.copy(atB[:, :], pv[cb][:, :])
                atB_sb.append(atB)
            # denominator: partition-reduce acc -> [1,512] -> 1/x -> rank-1
            # broadcast [128,512]; overlaps with the proj matmuls below
            dnr = ps_dn.tile([1, 512], F32, tag="dn", name="dnr")
            nc.tensor.matmul(dnr[:, :], lhsT=onesf_sb[:, :], rhs=acc[:, :],
                             start=True, stop=True)
            dnrec = dnr_pool.tile([1, 512], F32, tag="dnr", name="dnrec")
            nc.vector.reciprocal(dnrec[:, :], dnr[:, :])
            bcp = ps_dn.tile([128, 512], F32, tag="dn", name="bcp")
            nc.tensor.matmul(bcp[:, :], lhsT=onesrow_sb[:, :], rhs=dnrec[:, :],
                             start=True, stop=True)
            bc = bc_pool.tile([128, 512], F32, tag="bc", name="bc")
            nc.scalar.copy(bc[:, :], bcp[:, :])
            deferred = (atB_sb, bc, q0)
        emit_proj(*deferred)


def _build_main():
    nc = bacc.Bacc("TRN2", target_bir_lowering=False, debug=False,
                   num_devices=N_CORES)
    P = {}
    P["xf"] = nc.declare_dram_parameter("xf", [C, HW], F32, isOutput=False)
    for nm in ("wqT", "wkT", "wvT", "wpT"):
        P[nm] = nc.declare_dram_parameter(nm, [C, C], BF16, isOutput=False)
    for nm in ("bq2d", "bpe2d", "scl2d", "bia2d"):
        P[nm] = nc.declare_dram_parameter(nm, [128, CB], F32, isOutput=False)
    P["out"] = nc.declare_dram_parameter("out", [C, HALF], F32, isOutput=True)

    with tile.TileContext(nc) as tc:
        _body(tc, P)
    nc.finalize()
    return nc


def _get_ncs():
    if "nc" not in _CACHE:
        _CACHE["nc1"] = _build_stats()
        _CACHE["nc"] = _build_main()
    return _CACHE["nc1"], _CACHE["nc"]


def _frame_views(x):
    """Per-core rolled frame views: core i=(2f+h) sees frame f with its own
    half first."""
    views = []
    for i in range(N_CORES):
        f, h = divmod(i, 2)
        xfr = x[0, :, f].reshape(C, HW)
        if h == 1:
            xfr = np.concatenate([xfr[:, HALF:], xfr[:, :HALF]], axis=1)
        views.append(np.ascontiguousarray(xfr))
    return views


def _combine_stats(pstats_list, gamma, beta):
    """Host-side gather of kernel-1 partials -> per-channel scale/bias."""
    tot = np.zeros((128, 2 * CB), np.float64)
    for ps in pstats_list:
        tot += np.asarray(ps, np.float64)
    # column j holds channels [128j, 128j+128)
    s = tot[:, 0:CB].T.reshape(C)       # per-channel sum
    s2 = tot[:, CB:2 * CB].T.reshape(C)  # per-channel sumsq
    gs = s.reshape(G, C // G).sum(1)
    gs2 = s2.reshape(G, C // G).sum(1)
    meang = gs / NG_ELEMS
    varg = gs2 / NG_ELEMS - meang * meang
    rstd = 1.0 / np.sqrt(varg + EPS)
    chs = (np.asarray(gamma, np.float64) * np.repeat(rstd, C // G))
    chb = np.asarray(beta, np.float64) - np.repeat(meang, C // G) * chs
    def blk2d(v):
        return np.ascontiguousarray(v.astype(np.float32).reshape(CB, 128).T)
    return blk2d(chs), blk2d(chb)


def run_with_results(inputs, trace=False, **kw):
    bf16 = ml_dtypes.bfloat16
    f32 = np.float32
    x = np.asarray(inputs["x"], f32)
    gamma = np.asarray(inputs["gamma"], f32)
    beta = np.asarray(inputs["beta"], f32)
    wq, wk, wv, wp = [np.asarray(inputs[n], f32) for n in ("wq", "wk", "wv", "wp")]
    bq, bv, bp = [np.asarray(inputs[n], f32) for n in ("bq", "bv", "bp")]

    nc1, nc2 = _get_ncs()
    views = _frame_views(x)

    # ---- launch 1: partial GroupNorm stats over disjoint half-frames
    maps1 = [{"xh": views[i][:, :HALF]} for i in range(N_CORES)]
    maps1 = [{"xh": np.ascontiguousarray(m["xh"])} for m in maps1]
    res1 = run_bass_kernel_spmd(nc1, maps1, core_ids=list(range(N_CORES)),
                                trace=trace, **kw)
    scl2d, bia2d = _combine_stats([r["pstats"] for r in res1.results],
                                  gamma, beta)

    # ---- launch 2: the block itself
    def wT(w):
        return np.ascontiguousarray(w.T).astype(bf16)

    def blk2d(v):
        return np.ascontiguousarray(np.asarray(v, f32).reshape(CB, 128).T)

    shared = {
        "wqT": wT(wq), "wkT": wT(wk), "wvT": wT(wv), "wpT": wT(wp),
        "bq2d": blk2d(bq), "bpe2d": blk2d(bp + wp @ bv),
        "scl2d": scl2d, "bia2d": bia2d,
    }
    maps2 = [dict(shared, xf=views[i]) for i in range(N_CORES)]
    res2 = run_bass_kernel_spmd(nc2, maps2, core_ids=list(range(N_CORES)),
                                trace=trace, **kw)

    frames = []
    for f in range(T):
        a = np.asarray(res2.results[2 * f]["out"], dtype=np.float32)
        b = np.asarray(res2.results[2 * f + 1]["out"], dtype=np.float32)
        frames.append(np.concatenate([a, b], axis=1))
    out = np.stack(frames, axis=1)           # (C, T, HW)
    out = np.ascontiguousarray(out.reshape(1, C, T, 64, 64))
    return out, (res1, res2)


def kernel(**inputs):
    out, _ = run_with_results(inputs)
    return out



# revision 10
# speedup vs baseline: 1.4716x; 1.4716x over previous
"""GroupNorm + per-frame spatial attention block on 8 TRN2 NeuronCores.

Problem shape: x (1, 512, 4, 64, 64) f32.
  y   = GroupNorm32(x) (stats over (c/32, t, h, w) -> global over all frames)
  tok = y as (t, hw=4096, c=512)
  q,k,v = tok @ w{q,k,v}.T + b ; per-frame softmax(q k^T / sqrt(c)) v
  out = attn @ wp.T + bp ; return x + out

Sharding: core i handles frame f=i//2, query-half h=i%2 (2048 queries).
Each core redundantly computes K/V for its whole frame (cheaper than an
intra-pair all-gather).

Two launches (a fleet-wide collective barrier costs ~65us of latency, so
the tiny GroupNorm stats reduction is done as its own collective-free
kernel; the host combines the 8x[128,8] partial sums while "gathering"):
  kernel 1: per-core partial sum/sumsq over its disjoint half-frame.
  host:     combine partials -> per-channel scale/bias (512 numbers).
  kernel 2: normalize + qkv + attention + proj + residual.

All matmuls run in fp8e4 (TRN e4m3, max +-240) with DoubleRow perf mode:
one instruction contracts TWO 128-deep k-tiles (paired along dim1 of
[128, 2, N] tiles) at 2x bf16 throughput.  Scale management keeps every
fp8 operand in the format's sweet spot (validated on host: rel err vs
reference ~5.7e-3 against a 2e-2 gate):
  - weights are prescaled by WS=16 on the host (else ~27% of N(0,1/512)
    weight entries land in fp8 subnormals); undone by the 1/WS scale on
    the psum->sbuf activation copy.
  - p = exp(score/sqrt(c) - SHIFT), SHIFT=2: max p ~72 < 240, and the
    constant shift cancels exactly in the softmax normalization.
  - attention output is quantized unnormalized as pv/PRE, PRE=32 (max
    |pv| ~530); softmax denominator + PRE/WS are folded into the
    rank-1-broadcast normalization constant applied after the (linear)
    projection, so the PV psum banks free up immediately.

Math simplifications used (exact, not approximations):
  - bk drops out of softmax (adds a per-query constant to scores).
  - bv passes through attention unchanged (softmax weights sum to 1), so
    it is folded into the proj bias on the host: bp_eff = bp + wp @ bv.
  - the softmax denominator is the sum of the QUANTIZED p8 (ones-matmul
    on the PE, accumulated in psum alongside PV), so weights still sum
    to exactly 1 after normalization.
"""

import numpy as np
import ml_dtypes

import concourse.bass as bass
import concourse.bacc as bacc
import concourse.tile as tile
from concourse import mybir
from concourse.bass_utils import run_bass_kernel_spmd

C = 512
T = 4
HW = 64 * 64          # tokens per frame
HALF = HW // 2        # local queries per core
G = 32                # groups
N_CORES = 8
EPS = 1e-6
NG_ELEMS = (C // G) * T * HW   # elements per group in the full tensor
CB = C // 128         # 4 channel blocks
NP = CB // 2          # 2 channel-block pairs (DoubleRow k-tiles)
QG = HALF // 512      # 4 query groups of 512
NKT = HW // 128       # 32 key chunks of 128
NKP = NKT // 2        # 16 key chunk pairs
SCALE = float(C) ** -0.5
WS = 16.0             # host-side weight prescale (fp8 subnormal dodge)
SHIFT = 2.0           # exp shift: p = exp(s*SCALE - SHIFT), cancels in norm
PRE = 32.0            # attention-out prescale before fp8 quantization

E4NP = ml_dtypes.float8_e4m3   # TRN fp8e4 semantics (max +-240)

BF16 = mybir.dt.bfloat16
F32 = mybir.dt.float32
FP8 = mybir.dt.float8e4
AX = mybir.AxisListType
AF = mybir.ActivationFunctionType
OP = mybir.AluOpType
DR = mybir.MatmulPerfMode.DoubleRow

_CACHE = {}


# ---------------------------------------------------------------- kernel 1
def _build_stats():
    nc = bacc.Bacc("TRN2", target_bir_lowering=False, debug=False,
                   num_devices=N_CORES)
    xh = nc.declare_dram_parameter("xh", [C, HALF], F32, isOutput=False)
    pstats = nc.declare_dram_parameter("pstats", [128, 2 * CB], F32,
                                       isOutput=True)
    with tile.TileContext(nc) as tc:
        with tc.tile_pool(name="xt", bufs=CB) as xt_pool, \
             tc.tile_pool(name="scr", bufs=2) as scr_pool, \
             tc.tile_pool(name="st", bufs=1) as st_pool:
            # sums on DVE, sums-of-squares on ACT: the two run in parallel
            stats_sb = st_pool.tile([128, 2 * CB], F32, name="stats")
            stats2_sb = st_pool.tile([128, CB], F32, name="stats2")
            for j in range(CB):
                xt = xt_pool.tile([128, HALF], F32, tag="xt", name="xt")
                eng = nc.sync if j % 2 == 0 else nc.scalar
                eng.dma_start(xt[:, :], xh[j * 128:(j + 1) * 128, :])
                nc.vector.reduce_sum(stats_sb[:, j:j + 1], xt[:, :], axis=AX.X)
                scr = scr_pool.tile([128, HALF], F32, tag="scr", name="scr")
                nc.scalar.activation(scr[:, :], xt[:, :], AF.Square,
                                     accum_out=stats2_sb[:, j:j + 1])
            nc.vector.tensor_copy(stats_sb[:, CB:2 * CB], stats2_sb[:, :])
            nc.sync.dma_start(pstats[:, :], stats_sb[:, :])
    nc.finalize()
    return nc


# ---------------------------------------------------------------- kernel 2
def _body(tc, P):
    from contextlib import ExitStack

    nc = tc.nc
    with ExitStack() as ctx:
        consts = ctx.enter_context(tc.tile_pool(name="consts", bufs=1))

        # scale/bias ride the scalar HWDGE ring first (critical path for
        # the normalize), weights follow; the 8MB xf load rides sync.
        scl_sb = consts.tile([128, CB], F32, name="scl")
        nc.scalar.dma_start(scl_sb[:, :], P["scl2d"][:, :])
        bia_sb = consts.tile([128, CB], F32, name="bia")
        nc.scalar.dma_start(bia_sb[:, :], P["bia2d"][:, :])

        # local half of the frame stays resident: normalize source now,
        # residual read at proj time (saves the 4MB re-read).
        xloc = []
        for j in range(CB):
            xl = consts.tile([128, HALF], F32, name=f"xloc{j}")
            nc.sync.dma_start(xl[:, :], P["xf"][j * 128:(j + 1) * 128, 0:HALF])
            xloc.append(xl)

        def wtile(nm):
            t_ = consts.tile([128, 2, 2 * C], FP8, name=nm)
            nc.scalar.dma_start(t_[:, :, :], P[nm][:, :, :])
            return t_

        wq_sb = wtile("wq8")
        wk_sb = wtile("wk8")
        wv_sb = wtile("wv8")
        wp_sb = wtile("wp8")
        bq_sb = consts.tile([128, CB], F32, name="bq")
        nc.scalar.dma_start(bq_sb[:, :], P["bq2d"][:, :])
        bpe_sb = consts.tile([128, CB], F32, name="bpe")
        nc.scalar.dma_start(bpe_sb[:, :], P["bpe2d"][:, :])

        # [128, 2, 16] so the k-tile-pair stride is 16B (ISA: step%16==0);
        # only column 0 is used as the DoubleRow ones vector.
        ones8_sb = consts.tile([128, 2, 16], FP8, name="ones8")
        nc.vector.memset(ones8_sb[:, :, :], 1.0)
        onesrow_sb = consts.tile([1, 128], F32, name="onesrow")
        nc.vector.memset(onesrow_sb[:, :], PRE / WS)
        zero_sb = consts.tile([128, 1], F32, name="zero")
        nc.vector.memset(zero_sb[:, :], 0.0)
        nsh_sb = consts.tile([128, 1], F32, name="nsh")
        nc.vector.memset(nsh_sb[:, :], -SHIFT)

        # fp8 activations, channel-block-paired for DoubleRow
        xn_pool = ctx.enter_context(tc.tile_pool(name="xn", bufs=NP))
        xn_sb = [xn_pool.tile([128, 2, HW], FP8, tag="xn", name="xn")
                 for _ in range(NP)]
        q_pool = ctx.enter_context(tc.tile_pool(name="q", bufs=NP))
        q_sb = [q_pool.tile([128, 2, HALF], FP8, tag="q", name="q")
                for _ in range(NP)]
        k_pool = ctx.enter_context(tc.tile_pool(name="k", bufs=NP))
        k_sb = [k_pool.tile([128, 2, HW], FP8, tag="k", name="k")
                for _ in range(NP)]
        v_pool = ctx.enter_context(tc.tile_pool(name="v", bufs=NKP))
        v_sb = [v_pool.tile([128, 2, C], FP8, tag="v", name="v")
                for _ in range(NKP)]

        # psum pools: 4 + 2 + 1 + 1 = 8 banks
        ps_mm = ctx.enter_context(tc.tile_pool(name="ps_mm", bufs=4, space="PSUM"))
        ps_st = ctx.enter_context(tc.tile_pool(name="ps_st", bufs=2, space="PSUM"))
        ps_dn = ctx.enter_context(tc.tile_pool(name="ps_dn", bufs=1, space="PSUM"))
        ps_pp = ctx.enter_context(tc.tile_pool(name="ps_pp", bufs=1, space="PSUM"))

        p_pool = ctx.enter_context(tc.tile_pool(name="p", bufs=3))
        dnr_pool = ctx.enter_context(tc.tile_pool(name="dnr", bufs=2))
        bc_pool = ctx.enter_context(tc.tile_pool(name="bc", bufs=2))
        atB_pool = ctx.enter_context(tc.tile_pool(name="atB", bufs=4))
        ob_pool = ctx.enter_context(tc.tile_pool(name="ob", bufs=4))

        # normalize local half straight out of the resident tiles
        def norm(j, src, cs):
            nc.vector.tensor_scalar(
                out=xn_sb[j // 2][:, j % 2, cs], in0=src,
                scalar1=scl_sb[:, j:j + 1], scalar2=bia_sb[:, j:j + 1],
                op0=OP.mult, op1=OP.add)

        for j in range(CB):
            norm(j, xloc[j][:, :], slice(0, HALF))
        with tc.tile_pool(name="xf", bufs=3) as xf_pool:
            for j in range(CB):
                xt = xf_pool.tile([128, HALF], F32, tag="xf", name="xf")
                nc.sync.dma_start(xt[:, :], P["xf"][j * 128:(j + 1) * 128, HALF:HW])
                norm(j, xt[:, :], slice(HALF, HW))

        # ---------------- phase 1: q, k (channel-major), v (token-major) ----
        # Emit ALL work that only touches the first half of the frame
        # (q entirely + k/v first half) before anything needing the second
        # half: the second half's DMA+normalize is still in flight while
        # the PE chews through the first-half matmuls.
        def qk_group(w_sb, out_sb, j, t_, bias):
            ps = ps_mm.tile([128, 512], F32, tag="mm", name="mm")
            for ip in range(NP):
                nc.tensor.matmul(
                    ps[:, :],
                    lhsT=w_sb[:, :, ip * C + j * 128: ip * C + (j + 1) * 128],
                    rhs=xn_sb[ip][:, :, t_ * 512:(t_ + 1) * 512],
                    start=(ip == 0), stop=(ip == NP - 1), perf_mode=DR)
            dst = out_sb[j // 2][:, j % 2, t_ * 512:(t_ + 1) * 512]
            nc.scalar.activation(dst, ps[:, :], AF.Identity,
                                 scale=1.0 / WS, bias=bias)

        def v_group(m):
            ps = ps_mm.tile([128, 512], F32, tag="mm", name="mm")
            for ip in range(NP):
                nc.tensor.matmul(
                    ps[:, :],
                    lhsT=xn_sb[ip][:, :, m * 128:(m + 1) * 128],
                    rhs=wv_sb[:, :, ip * C:(ip + 1) * C],
                    start=(ip == 0), stop=(ip == NP - 1), perf_mode=DR)
            nc.vector.tensor_scalar(out=v_sb[m // 2][:, m % 2, :], in0=ps[:, :],
                                    scalar1=1.0 / WS, scalar2=None, op0=OP.mult)

        for j in range(CB):          # q covers exactly the first half
            for t_ in range(QG):
                qk_group(wq_sb, q_sb, j, t_, bias=bq_sb[:, j:j + 1])
        for j in range(CB):          # k, first half
            for t_ in range(4):
                qk_group(wk_sb, k_sb, j, t_, bias=zero_sb[:, :])
        for m in range(NKT // 2):    # v, first half
            v_group(m)
        for j in range(CB):          # k, second half
            for t_ in range(4, 8):
                qk_group(wk_sb, k_sb, j, t_, bias=zero_sb[:, :])
        for m in range(NKT // 2, NKT):
            v_group(m)

        # ---------------- phase 2: attention + proj per query group --------
        # proj of group g is emitted at the START of group g+1: its matmuls
        # are ready instantly (own psum bank, inputs done) and fill the PE
        # window where the next score matmuls wait on the exp lag.
        def emit_proj(atB_sb, bc, q0):
            for cb in range(CB):
                pp = ps_pp.tile([128, 512], F32, tag="pp", name="pp")
                for ip in range(NP):
                    nc.tensor.matmul(
                        pp[:, :],
                        lhsT=wp_sb[:, :, ip * C + cb * 128: ip * C + (cb + 1) * 128],
                        rhs=atB_sb[ip][:, :, :],
                        start=(ip == 0), stop=(ip == NP - 1), perf_mode=DR)
                t1 = ob_pool.tile([128, 512], F32, tag="t1", name="t1")
                nc.vector.tensor_mul(t1[:, :], pp[:, :], bc[:, :])
                ob = ob_pool.tile([128, 512], F32, tag="ob", name="ob")
                nc.vector.scalar_tensor_tensor(
                    ob[:, :], in0=t1[:, :], scalar=bpe_sb[:, cb:cb + 1],
                    in1=xloc[cb][:, q0:q0 + 512], op0=OP.add, op1=OP.add)
                nc.sync.dma_start(P["out"][cb * 128:(cb + 1) * 128, q0:q0 + 512],
                                  ob[:, :])

        deferred = None
        for qg in range(QG):
            q0 = qg * 512
            pv = [ps_mm.tile([128, 512], F32, tag="mm", name="mm")
                  for _ in range(CB)]
            dnr = ps_dn.tile([1, 512], F32, tag="dn", name="dnr")
            if deferred is not None:
                emit_proj(*deferred)
                deferred = None
            for m2 in range(NKP):
                p8 = p_pool.tile([128, 2, 512], FP8, tag="p", name="p")
                for mm in range(2):
                    m = 2 * m2 + mm
                    st = ps_st.tile([128, 512], F32, tag="st", name="st")
                    for ip in range(NP):
                        nc.tensor.matmul(
                            st[:, :],
                            lhsT=k_sb[ip][:, :, m * 128:(m + 1) * 128],
                            rhs=q_sb[ip][:, :, q0:q0 + 512],
                            start=(ip == 0), stop=(ip == NP - 1), perf_mode=DR)
                    nc.scalar.activation(p8[:, mm, :], st[:, :], AF.Exp,
                                         scale=SCALE, bias=nsh_sb[:, :])
                # denominator rides the PE: dnr[0,q] += sum_kt p8[kt,:,q]
                nc.tensor.matmul(dnr[:, :], lhsT=ones8_sb[:, :, 0:1],
                                 rhs=p8[:, :, :],
                                 start=(m2 == 0), stop=(m2 == NKP - 1),
                                 perf_mode=DR)
                for cb in range(CB):
                    # attention output channel-major: out[co, qt] += v^T p
                    nc.tensor.matmul(
                        pv[cb][:, :],
                        lhsT=v_sb[m2][:, :, cb * 128:(cb + 1) * 128],
                        rhs=p8[:, :, :],
                        start=(m2 == 0), stop=(m2 == NKP - 1), perf_mode=DR)
            # quantize UNNORMALIZED attention out of PSUM right away (frees
            # the pv banks for the next query group); denominator + scales
            # are applied after the (linear) projection instead.
            atB_sb = []
            for pr in range(NP):
                atB = atB_pool.tile([128, 2, 512], FP8, tag="atB", name="atB")
                for i in range(2):
                    nc.scalar.activation(atB[:, i, :], pv[2 * pr + i][:, :],
                                         AF.Identity, scale=1.0 / PRE,
                                         bias=zero_sb[:, :])
                atB_sb.append(atB)
            # bc = (PRE/WS) / denominator, rank-1-broadcast on the PE;
            # overlaps with the next group's score matmuls
            dnrec = dnr_pool.tile([1, 512], F32, tag="dnr", name="dnrec")
            nc.vector.reciprocal(dnrec[:, :], dnr[:, :])
            bcp = ps_pp.tile([128, 512], F32, tag="pp", name="bcp")
            nc.tensor.matmul(bcp[:, :], lhsT=onesrow_sb[:, :], rhs=dnrec[:, :],
                             start=True, stop=True)
            bc = bc_pool.tile([128, 512], F32, tag="bc", name="bc")
            nc.scalar.copy(bc[:, :], bcp[:, :])
            deferred = (atB_sb, bc, q0)
        emit_proj(*deferred)


def _build_main():
    nc = bacc.Bacc("TRN2", target_bir_lowering=False, debug=False,
                   num_devices=N_CORES)
    P = {}
    P["xf"] = nc.declare_dram_parameter("xf", [C, HW], F32, isOutput=False)
    for nm in ("wq8", "wk8", "wv8", "wp8"):
        P[nm] = nc.declare_dram_parameter(nm, [128, 2, 2 * C], FP8,
                                          isOutput=False)
    for nm in ("bq2d", "bpe2d", "scl2d", "bia2d"):
        P[nm] = nc.declare_dram_parameter(nm, [128, CB], F32, isOutput=False)
    P["out"] = nc.declare_dram_parameter("out", [C, HALF], F32, isOutput=True)

    with tile.TileContext(nc) as tc:
        _body(tc, P)
    nc.finalize()
    return nc


def _get_ncs():
    if "nc" not in _CACHE:
        _CACHE["nc1"] = _build_stats()
        _CACHE["nc"] = _build_main()
    return _CACHE["nc1"], _CACHE["nc"]


def _frame_views(x):
    """Per-core rolled frame views: core i=(2f+h) sees frame f with its own
    half first."""
    views = []
    for i in range(N_CORES):
        f, h = divmod(i, 2)
        xfr = x[0, :, f].reshape(C, HW)
        if h == 1:
            xfr = np.concatenate([xfr[:, HALF:], xfr[:, :HALF]], axis=1)
        views.append(np.ascontiguousarray(xfr))
    return views


def _combine_stats(pstats_list, gamma, beta):
    """Host-side gather of kernel-1 partials -> per-channel scale/bias."""
    tot = np.zeros((128, 2 * CB), np.float64)
    for ps in pstats_list:
        tot += np.asarray(ps, np.float64)
    # column j holds channels [128j, 128j+128)
    s = tot[:, 0:CB].T.reshape(C)       # per-channel sum
    s2 = tot[:, CB:2 * CB].T.reshape(C)  # per-channel sumsq
    gs = s.reshape(G, C // G).sum(1)
    gs2 = s2.reshape(G, C // G).sum(1)
    meang = gs / NG_ELEMS
    varg = gs2 / NG_ELEMS - meang * meang
    rstd = 1.0 / np.sqrt(varg + EPS)
    chs = (np.asarray(gamma, np.float64) * np.repeat(rstd, C // G))
    chb = np.asarray(beta, np.float64) - np.repeat(meang, C // G) * chs
    def blk2d(v):
        return np.ascontiguousarray(v.astype(np.float32).reshape(CB, 128).T)
    return blk2d(chs), blk2d(chb)


def _w8pack(w):
    """(c_out, c_in) f32 -> [128, 2, 2C] fp8e4, channel-block-pair packed:
    out[p, i, ip*C + o] = w.T[(2*ip + i)*128 + p, o] * WS."""
    a = (np.asarray(w, np.float32).T * WS).reshape(2, 2, 128, C)
    a = np.ascontiguousarray(a.transpose(2, 1, 0, 3).reshape(128, 2, 2 * C))
    return a.astype(E4NP)


def run_with_results(inputs, trace=False, **kw):
    f32 = np.float32
    x = np.asarray(inputs["x"], f32)
    gamma = np.asarray(inputs["gamma"], f32)
    beta = np.asarray(inputs["beta"], f32)
    wq, wk, wv, wp = [np.asarray(inputs[n], f32) for n in ("wq", "wk", "wv", "wp")]
    bq, bv, bp = [np.asarray(inputs[n], f32) for n in ("bq", "bv", "bp")]

    nc1, nc2 = _get_ncs()
    views = _frame_views(x)

    # ---- launch 1: partial GroupNorm stats over disjoint half-frames
    maps1 = [{"xh": np.ascontiguousarray(views[i][:, :HALF])}
             for i in range(N_CORES)]
    res1 = run_bass_kernel_spmd(nc1, maps1, core_ids=list(range(N_CORES)),
                                trace=trace, **kw)
    scl2d, bia2d = _combine_stats([r["pstats"] for r in res1.results],
                                  gamma, beta)

    # ---- launch 2: the block itself
    def blk2d(v):
        return np.ascontiguousarray(np.asarray(v, f32).reshape(CB, 128).T)

    shared = {
        "wq8": _w8pack(wq), "wk8": _w8pack(wk), "wv8": _w8pack(wv),
        "wp8": _w8pack(wp),
        "bq2d": blk2d(bq), "bpe2d": blk2d(bp + wp @ bv),
        "scl2d": scl2d, "bia2d": bia2d,
    }
    maps2 = [dict(shared, xf=views[i]) for i in range(N_CORES)]
    res2 = run_bass_kernel_spmd(nc2, maps2, core_ids=list(range(N_CORES)),
                                trace=trace, **kw)

    frames = []
    for f in range(T):
        a = np.asarray(res2.results[2 * f]["out"], dtype=np.float32)
        b = np.asarray(res2.results[2 * f + 1]["out"], dtype=np.float32)
        frames.append(np.concatenate([a, b], axis=1))
    out = np.stack(frames, axis=1)           # (C, T, HW)
    out = np.ascontiguousarray(out.reshape(1, C, T, 64, 64))
    return out, (res1, res2)


def kernel(**inputs):
    out, _ = run_with_results(inputs)
    return out


# revision 20
# speedup vs baseline: 1.5696x; 1.0666x over previous
"""GroupNorm + per-frame spatial attention block on 8 TRN2 NeuronCores.

Problem shape: x (1, 512, 4, 64, 64) f32.
  y   = GroupNorm32(x) (stats over (c/32, t, h, w) -> global over all frames)
  tok = y as (t, hw=4096, c=512)
  q,k,v = tok @ w{q,k,v}.T + b ; per-frame softmax(q k^T / sqrt(c)) v
  out = attn @ wp.T + bp ; return x + out

Sharding: core i handles frame f=i//2, query-half h=i%2 (2048 queries).
Each core redundantly computes K/V for its whole frame (cheaper than an
intra-pair all-gather).

Two launches (a fleet-wide collective barrier costs ~65us of latency, so
the tiny GroupNorm stats reduction is done as its own collective-free
kernel; the host combines the 8x[128,8] partial sums while "gathering"):
  kernel 1: per-core partial sum/sumsq over its disjoint half-frame.
  host:     combine partials -> per-channel scale/bias (512 numbers).
  kernel 2: normalize + qkv + attention + proj + residual.

All matmuls run in fp8e4 (TRN e4m3, max +-240) with DoubleRow perf mode:
one instruction contracts TWO 128-deep k-tiles (paired along dim1 of
[128, 2, N] tiles) at 2x bf16 throughput.  Scale management keeps every
fp8 operand in the format's sweet spot (validated on host: rel err vs
reference ~5.7e-3 against a 2e-2 gate):
  - weights are prescaled by WS=16 on the host (else ~27% of N(0,1/512)
    weight entries land in fp8 subnormals); undone by the 1/WS scale on
    the psum->sbuf activation copy.
  - p = exp(score/sqrt(c) - SHIFT), SHIFT=2: max p ~72 < 240, and the
    constant shift cancels exactly in the softmax normalization.
  - attention output is quantized unnormalized as pv/PRE, PRE=32 (max
    |pv| ~530); softmax denominator + PRE/WS are folded into the
    rank-1-broadcast normalization constant applied after the (linear)
    projection, so the PV psum banks free up immediately.

Math simplifications used (exact, not approximations):
  - bk drops out of softmax (adds a per-query constant to scores).
  - bv passes through attention unchanged (softmax weights sum to 1), so
    it is folded into the proj bias on the host: bp_eff = bp + wp @ bv.
  - the softmax denominator is the sum of the QUANTIZED p8 (ones-matmul
    on the PE, accumulated in psum alongside PV), so weights still sum
    to exactly 1 after normalization.
"""

import numpy as np
import ml_dtypes

import concourse.bass as bass
import concourse.bacc as bacc
import concourse.tile as tile
from concourse import mybir
from concourse.bass_utils import run_bass_kernel_spmd

C = 512
T = 4
HW = 64 * 64          # tokens per frame
HALF = HW // 2        # local queries per core
G = 32                # groups
N_CORES = 8
EPS = 1e-6
NG_ELEMS = (C // G) * T * HW   # elements per group in the full tensor
CB = C // 128         # 4 channel blocks
NP = CB // 2          # 2 channel-block pairs (DoubleRow k-tiles)
QG = HALF // 512      # 4 query groups of 512
NKT = HW // 128       # 32 key chunks of 128
NKP = NKT // 2        # 16 key chunk pairs
SCALE = float(C) ** -0.5
WS = 16.0             # host-side weight prescale (fp8 subnormal dodge)
SHIFT = 2.0           # exp shift: p = exp(s*SCALE - SHIFT), cancels in norm
PRE = 32.0            # attention-out prescale before fp8 quantization

E4NP = ml_dtypes.float8_e4m3   # TRN fp8e4 semantics (max +-240)

BF16 = mybir.dt.bfloat16
F32 = mybir.dt.float32
FP8 = mybir.dt.float8e4
AX = mybir.AxisListType
AF = mybir.ActivationFunctionType
OP = mybir.AluOpType
DR = mybir.MatmulPerfMode.DoubleRow

_CACHE = {}


# ---------------------------------------------------------------- kernel 1
NCH = 8                   # stat chunks: 2 per channel block (bf16 input)
CHTOK = HALF // 2         # tokens per stat chunk


def _build_stats():
    """Partial sum/sumsq over this core's half-frame (bf16 input halves the
    DMA).  8 chunks of [128, 1024]; sums split DVE/ACT, squares on ACT.
    pstats col c = chunk-c sum, col 8+c = chunk-c sumsq (host combines)."""
    nc = bacc.Bacc("TRN2", target_bir_lowering=False, debug=False,
                   num_devices=N_CORES)
    xh = nc.declare_dram_parameter("xh", [C, HALF], BF16, isOutput=False)
    pstats = nc.declare_dram_parameter("pstats", [128, 2 * NCH], F32,
                                       isOutput=True)
    with tile.TileContext(nc) as tc:
        with tc.tile_pool(name="xt", bufs=NCH) as xt_pool, \
             tc.tile_pool(name="scr", bufs=3) as scr_pool, \
             tc.tile_pool(name="st", bufs=1) as st_pool:
            stats_sb = st_pool.tile([128, 2 * NCH], F32, name="stats")
            for c in range(NCH):
                j, h = divmod(c, 2)
                xt = xt_pool.tile([128, CHTOK], BF16, tag="xt", name="xt")
                eng = nc.sync if c % 2 == 0 else nc.scalar
                eng.dma_start(xt[:, :], xh[j * 128:(j + 1) * 128,
                                           h * CHTOK:(h + 1) * CHTOK])
                if c not in (3, 7):
                    nc.vector.reduce_sum(stats_sb[:, c:c + 1], xt[:, :],
                                         axis=AX.X)
                else:
                    scr = scr_pool.tile([128, CHTOK], F32, tag="scr", name="s0")
                    nc.scalar.activation(scr[:, :], xt[:, :], AF.Identity,
                                         accum_out=stats_sb[:, c:c + 1])
                scr = scr_pool.tile([128, CHTOK], F32, tag="scr", name="scr")
                nc.scalar.activation(scr[:, :], xt[:, :], AF.Square,
                                     accum_out=stats_sb[:, NCH + c:NCH + c + 1])
            nc.sync.dma_start(pstats[:, :], stats_sb[:, :])
    nc.finalize()
    return nc


# ---------------------------------------------------------------- kernel 2
def _body(tc, P):
    from contextlib import ExitStack

    nc = tc.nc
    with ExitStack() as ctx:
        consts = ctx.enter_context(tc.tile_pool(name="consts", bufs=1))

        # scale/bias ride the scalar HWDGE ring first (critical path for
        # the normalize), weights follow; the 8MB xf load rides sync.
        scl_sb = consts.tile([128, CB], F32, name="scl")
        nc.scalar.dma_start(scl_sb[:, :], P["scl2d"][:, :])
        bia_sb = consts.tile([128, CB], F32, name="bia")
        nc.scalar.dma_start(bia_sb[:, :], P["bia2d"][:, :])

        # local half of the frame stays resident: normalize source now,
        # residual read at proj time (saves the 4MB re-read).  DMA'd in
        # 512-token column groups below so the PE starts early.
        xloc = [consts.tile([128, HALF], F32, name=f"xloc{j}")
                for j in range(CB)]

        def wtile(nm):
            t_ = consts.tile([128, 2, 2 * C], FP8, name=nm)
            nc.scalar.dma_start(t_[:, :, :], P[nm][:, :, :])
            return t_

        wq_sb = wtile("wq8")
        wk_sb = wtile("wk8")
        wv_sb = wtile("wv8")
        wp_sb = wtile("wp8")
        bq_sb = consts.tile([128, CB], F32, name="bq")
        nc.scalar.dma_start(bq_sb[:, :], P["bq2d"][:, :])
        bpe_sb = consts.tile([128, CB], F32, name="bpe")
        nc.scalar.dma_start(bpe_sb[:, :], P["bpe2d"][:, :])

        # [128, 2, 16] so the k-tile-pair stride is 16B (ISA: step%16==0);
        # only column 0 is used as the DoubleRow ones vector.
        ones8_sb = consts.tile([128, 2, 16], FP8, name="ones8")
        nc.vector.memset(ones8_sb[:, :, :], 1.0)
        onesrow_sb = consts.tile([1, 128], BF16, name="onesrow")
        nc.vector.memset(onesrow_sb[:, :], PRE / WS)
        zero_sb = consts.tile([128, 1], F32, name="zero")
        nc.vector.memset(zero_sb[:, :], 0.0)
        nsh_sb = consts.tile([128, 1], F32, name="nsh")
        nc.vector.memset(nsh_sb[:, :], -SHIFT)

        # fp8 activations, channel-block-paired for DoubleRow
        xn_pool = ctx.enter_context(tc.tile_pool(name="xn", bufs=NP))
        xn_sb = [xn_pool.tile([128, 2, HW], FP8, tag="xn", name="xn")
                 for _ in range(NP)]
        q_pool = ctx.enter_context(tc.tile_pool(name="q", bufs=NP))
        q_sb = [q_pool.tile([128, 2, HALF], FP8, tag="q", name="q")
                for _ in range(NP)]
        k_pool = ctx.enter_context(tc.tile_pool(name="k", bufs=NP))
        k_sb = [k_pool.tile([128, 2, HW], FP8, tag="k", name="k")
                for _ in range(NP)]
        v_pool = ctx.enter_context(tc.tile_pool(name="v", bufs=NKP))
        v_sb = [v_pool.tile([128, 2, C], FP8, tag="v", name="v")
                for _ in range(NKP)]

        # psum pools: 4 + 2 + 1 + 1 = 8 banks
        ps_mm = ctx.enter_context(tc.tile_pool(name="ps_mm", bufs=4, space="PSUM"))
        ps_st = ctx.enter_context(tc.tile_pool(name="ps_st", bufs=2, space="PSUM"))
        ps_dn = ctx.enter_context(tc.tile_pool(name="ps_dn", bufs=1, space="PSUM"))
        ps_pp = ctx.enter_context(tc.tile_pool(name="ps_pp", bufs=1, space="PSUM"))

        p_pool = ctx.enter_context(tc.tile_pool(name="p", bufs=3))
        dnr_pool = ctx.enter_context(tc.tile_pool(name="dnr", bufs=2))
        bc_pool = ctx.enter_context(tc.tile_pool(name="bc", bufs=2))
        atB_pool = ctx.enter_context(tc.tile_pool(name="atB", bufs=4))
        ob_pool = ctx.enter_context(tc.tile_pool(name="ob", bufs=4))

        # ---------------- phase 0+1: streamed normalize + q/k/v -------------
        # token-group-major streaming: per 512-token group, DMA + normalize
        # its 4 channel blocks, then immediately emit every matmul that only
        # needs tokens seen so far; the PE starts ~10us earlier than with
        # half-frame-granular loads.
        def norm(j, src, cs):
            nc.vector.tensor_scalar(
                out=xn_sb[j // 2][:, j % 2, cs], in0=src,
                scalar1=scl_sb[:, j:j + 1], scalar2=bia_sb[:, j:j + 1],
                op0=OP.mult, op1=OP.add)

        def qk_group(w_sb, out_sb, j, t_, bias):
            ps = ps_mm.tile([128, 512], F32, tag="mm", name="mm")
            for ip in range(NP):
                nc.tensor.matmul(
                    ps[:, :],
                    lhsT=w_sb[:, :, ip * C + j * 128: ip * C + (j + 1) * 128],
                    rhs=xn_sb[ip][:, :, t_ * 512:(t_ + 1) * 512],
                    start=(ip == 0), stop=(ip == NP - 1), perf_mode=DR)
            dst = out_sb[j // 2][:, j % 2, t_ * 512:(t_ + 1) * 512]
            nc.scalar.activation(dst, ps[:, :], AF.Identity,
                                 scale=1.0 / WS, bias=bias)

        def v_group(m):
            ps = ps_mm.tile([128, 512], F32, tag="mm", name="mm")
            for ip in range(NP):
                nc.tensor.matmul(
                    ps[:, :],
                    lhsT=xn_sb[ip][:, :, m * 128:(m + 1) * 128],
                    rhs=wv_sb[:, :, ip * C:(ip + 1) * C],
                    start=(ip == 0), stop=(ip == NP - 1), perf_mode=DR)
            nc.vector.tensor_scalar(out=v_sb[m // 2][:, m % 2, :], in0=ps[:, :],
                                    scalar1=1.0 / WS, scalar2=None, op0=OP.mult)

        with tc.tile_pool(name="xf", bufs=8) as xf_pool:
            for tg in range(8):
                ts_, te_ = tg * 512, (tg + 1) * 512
                for j in range(CB):
                    if tg < QG:      # local half: land in the resident tiles
                        dst = xloc[j][:, ts_:te_]
                    else:
                        xt = xf_pool.tile([128, 512], F32, tag="xf", name="xf")
                        dst = xt[:, :]
                    nc.sync.dma_start(dst, P["xf"][j * 128:(j + 1) * 128,
                                                   ts_:te_])
                    norm(j, dst, slice(ts_, te_))
                if tg < QG:          # q covers exactly the local half
                    for j in range(CB):
                        qk_group(wq_sb, q_sb, j, tg, bias=bq_sb[:, j:j + 1])
                for j in range(CB):
                    qk_group(wk_sb, k_sb, j, tg, bias=zero_sb[:, :])
                for m in range(4 * tg, 4 * tg + 4):
                    v_group(m)

        # ---------------- phase 2: attention + proj per query group --------
        # proj of group g is emitted at the START of group g+1: its matmuls
        # are ready instantly (own psum bank, inputs done) and fill the PE
        # window where the next score matmuls wait on the exp lag.
        def emit_proj(atB_sb, bc, q0, pool):
            for cb in range(CB):
                pp = pool.tile([128, 512], F32, tag="mm" if pool is ps_mm
                               else "pp", name="pp")
                for ip in range(NP):
                    nc.tensor.matmul(
                        pp[:, :],
                        lhsT=wp_sb[:, :, ip * C + cb * 128: ip * C + (cb + 1) * 128],
                        rhs=atB_sb[ip][:, :, :],
                        start=(ip == 0), stop=(ip == NP - 1), perf_mode=DR)
                t1 = ob_pool.tile([128, 512], F32, tag="t1", name="t1")
                nc.vector.tensor_mul(t1[:, :], pp[:, :], bc[:, :])
                ob = ob_pool.tile([128, 512], F32, tag="ob", name="ob")
                nc.vector.scalar_tensor_tensor(
                    ob[:, :], in0=t1[:, :], scalar=bpe_sb[:, cb:cb + 1],
                    in1=xloc[cb][:, q0:q0 + 512], op0=OP.add, op1=OP.add)
                nc.sync.dma_start(P["out"][cb * 128:(cb + 1) * 128, q0:q0 + 512],
                                  ob[:, :])

        deferred = None
        for qg in range(QG):
            q0 = qg * 512
            pv = [ps_mm.tile([128, 512], F32, tag="mm", name="mm")
                  for _ in range(CB)]
            dnr = ps_dn.tile([1, 512], F32, tag="dn", name="dnr")
            if deferred is not None:
                emit_proj(*deferred, pool=ps_pp)
                deferred = None
            for m2 in range(NKP):
                p8 = p_pool.tile([128, 2, 512], FP8, tag="p", name="p")
                for mm in range(2):
                    m = 2 * m2 + mm
                    st = ps_st.tile([128, 512], F32, tag="st", name="st")
                    for ip in range(NP):
                        nc.tensor.matmul(
                            st[:, :],
                            lhsT=k_sb[ip][:, :, m * 128:(m + 1) * 128],
                            rhs=q_sb[ip][:, :, q0:q0 + 512],
                            start=(ip == 0), stop=(ip == NP - 1), perf_mode=DR)
                    nc.scalar.activation(p8[:, mm, :], st[:, :], AF.Exp,
                                         scale=SCALE, bias=nsh_sb[:, :])
                # denominator rides the PE: dnr[0,q] += sum_kt p8[kt,:,q]
                nc.tensor.matmul(dnr[:, :], lhsT=ones8_sb[:, :, 0:1],
                                 rhs=p8[:, :, :],
                                 start=(m2 == 0), stop=(m2 == NKP - 1),
                                 perf_mode=DR)
                for cb in range(CB):
                    # attention output channel-major: out[co, qt] += v^T p
                    nc.tensor.matmul(
                        pv[cb][:, :],
                        lhsT=v_sb[m2][:, :, cb * 128:(cb + 1) * 128],
                        rhs=p8[:, :, :],
                        start=(m2 == 0), stop=(m2 == NKP - 1), perf_mode=DR)
            # quantize UNNORMALIZED attention out of PSUM right away (frees
            # the pv banks for the next query group); denominator + scales
            # are applied after the (linear) projection instead.
            atB_sb = []
            for pr in range(NP):
                atB = atB_pool.tile([128, 2, 512], FP8, tag="atB", name="atB")
                for i in range(2):
                    nc.scalar.activation(atB[:, i, :], pv[2 * pr + i][:, :],
                                         AF.Identity, scale=1.0 / PRE,
                                         bias=zero_sb[:, :])
                atB_sb.append(atB)
            # bc = (PRE/WS) / denominator, rank-1-broadcast on the PE;
            # overlaps with the next group's score matmuls.  bf16 keeps the
            # broadcast matmul at 1 cyc/row (f32 is 4); the ~0.2% rounding
            # on 1/D is far below the gate.
            dnrec = dnr_pool.tile([1, 512], BF16, tag="dnr", name="dnrec")
            with nc.allow_low_precision("bf16 1/denominator: 0.4% on a "
                                        "2e-2-gated output"):
                nc.vector.reciprocal(dnrec[:, :], dnr[:, :])
            bcp = ps_pp.tile([128, 512], F32, tag="pp", name="bcp")
            nc.tensor.matmul(bcp[:, :], lhsT=onesrow_sb[:, :], rhs=dnrec[:, :],
                             start=True, stop=True)
            bc = bc_pool.tile([128, 512], F32, tag="bc", name="bc")
            nc.scalar.copy(bc[:, :], bcp[:, :])
            deferred = (atB_sb, bc, q0)
        emit_proj(*deferred, pool=ps_mm)


def _build_main():
    nc = bacc.Bacc("TRN2", target_bir_lowering=False, debug=False,
                   num_devices=N_CORES)
    P = {}
    P["xf"] = nc.declare_dram_parameter("xf", [C, HW], F32, isOutput=False)
    for nm in ("wq8", "wk8", "wv8", "wp8"):
        P[nm] = nc.declare_dram_parameter(nm, [128, 2, 2 * C], FP8,
                                          isOutput=False)
    for nm in ("bq2d", "bpe2d", "scl2d", "bia2d"):
        P[nm] = nc.declare_dram_parameter(nm, [128, CB], F32, isOutput=False)
    P["out"] = nc.declare_dram_parameter("out", [C, HALF], F32, isOutput=True)

    with tile.TileContext(nc) as tc:
        _body(tc, P)
    nc.finalize()
    return nc


def _get_ncs():
    if "nc" not in _CACHE:
        _CACHE["nc1"] = _build_stats()
        _CACHE["nc"] = _build_main()
    return _CACHE["nc1"], _CACHE["nc"]


def _frame_views(x):
    """Per-core rolled frame views: core i=(2f+h) sees frame f with its own
    half first."""
    views = []
    for i in range(N_CORES):
        f, h = divmod(i, 2)
        xfr = x[0, :, f].reshape(C, HW)
        if h == 1:
            xfr = np.concatenate([xfr[:, HALF:], xfr[:, :HALF]], axis=1)
        views.append(np.ascontiguousarray(xfr))
    return views


def _combine_stats(pstats_list, gamma, beta):
    """Host-side gather of kernel-1 partials -> per-channel scale/bias."""
    tot = np.zeros((128, 2 * NCH), np.float64)
    for ps in pstats_list:
        tot += np.asarray(ps, np.float64)
    # chunk c covers channel block c//2: fold the two token-halves
    sc = tot[:, 0:NCH].reshape(128, CB, 2).sum(2)       # [128, CB] sums
    sc2 = tot[:, NCH:2 * NCH].reshape(128, CB, 2).sum(2)
    s = sc.T.reshape(C)       # per-channel sum
    s2 = sc2.T.reshape(C)     # per-channel sumsq
    gs = s.reshape(G, C // G).sum(1)
    gs2 = s2.reshape(G, C // G).sum(1)
    meang = gs / NG_ELEMS
    varg = gs2 / NG_ELEMS - meang * meang
    rstd = 1.0 / np.sqrt(varg + EPS)
    chs = (np.asarray(gamma, np.float64) * np.repeat(rstd, C // G))
    chb = np.asarray(beta, np.float64) - np.repeat(meang, C // G) * chs
    def blk2d(v):
        return np.ascontiguousarray(v.astype(np.float32).reshape(CB, 128).T)
    return blk2d(chs), blk2d(chb)


def _w8pack(w):
    """(c_out, c_in) f32 -> [128, 2, 2C] fp8e4, channel-block-pair packed:
    out[p, i, ip*C + o] = w.T[(2*ip + i)*128 + p, o] * WS."""
    a = (np.asarray(w, np.float32).T * WS).reshape(2, 2, 128, C)
    a = np.ascontiguousarray(a.transpose(2, 1, 0, 3).reshape(128, 2, 2 * C))
    return a.astype(E4NP)


def run_with_results(inputs, trace=False, **kw):
    f32 = np.float32
    x = np.asarray(inputs["x"], f32)
    gamma = np.asarray(inputs["gamma"], f32)
    beta = np.asarray(inputs["beta"], f32)
    wq, wk, wv, wp = [np.asarray(inputs[n], f32) for n in ("wq", "wk", "wv", "wp")]
    bq, bv, bp = [np.asarray(inputs[n], f32) for n in ("bq", "bv", "bp")]

    nc1, nc2 = _get_ncs()
    views = _frame_views(x)

    # ---- launch 1: partial GroupNorm stats over disjoint half-frames
    # (bf16 input: halves the DMA; the stats shift is far below the gate)
    maps1 = [{"xh": np.ascontiguousarray(views[i][:, :HALF])
              .astype(ml_dtypes.bfloat16)}
             for i in range(N_CORES)]
    res1 = run_bass_kernel_spmd(nc1, maps1, core_ids=list(range(N_CORES)),
                                trace=trace, **kw)
    scl2d, bia2d = _combine_stats([r["pstats"] for r in res1.results],
                                  gamma, beta)

    # ---- launch 2: the block itself
    def blk2d(v):
        return np.ascontiguousarray(np.asarray(v, f32).reshape(CB, 128).T)

    shared = {
        "wq8": _w8pack(wq), "wk8": _w8pack(wk), "wv8": _w8pack(wv),
        "wp8": _w8pack(wp),
        "bq2d": blk2d(bq), "bpe2d": blk2d(bp + wp @ bv),
        "scl2d": scl2d, "bia2d": bia2d,
    }
    maps2 = [dict(shared, xf=views[i]) for i in range(N_CORES)]
    res2 = run_bass_kernel_spmd(nc2, maps2, core_ids=list(range(N_CORES)),
                                trace=trace, **kw)

    frames = []
    for f in range(T):
        a = np.asarray(res2.results[2 * f]["out"], dtype=np.float32)
        b = np.asarray(res2.results[2 * f + 1]["out"], dtype=np.float32)
        frames.append(np.concatenate([a, b], axis=1))
    out = np.stack(frames, axis=1)           # (C, T, HW)
    out = np.ascontiguousarray(out.reshape(1, C, T, 64, 64))
    return out, (res1, res2)


def kernel(**inputs):
    out, _ = run_with_results(inputs)
    return out


# revision 26
# speedup vs baseline: 1.7093x; 1.0890x over previous
"""GroupNorm + per-frame spatial attention block on 8 TRN2 NeuronCores.

Problem shape: x (1, 512, 4, 64, 64) f32.
  y   = GroupNorm32(x) (stats over (c/32, t, h, w) -> global over all frames)
  tok = y as (t, hw=4096, c=512)
  q,k,v = tok @ w{q,k,v}.T + b ; per-frame softmax(q k^T / sqrt(c)) v
  out = attn @ wp.T + bp ; return x + out

Sharding: core i handles frame f=i//2, query-half h=i%2 (2048 queries).
Each core redundantly computes K/V for its whole frame (cheaper than an
intra-pair all-gather).

Two launches (a fleet-wide collective barrier costs ~65us of latency, so
the tiny GroupNorm stats reduction is done as its own collective-free
kernel; the host combines the 8x[128,8] partial sums while "gathering"):
  kernel 1: per-core partial sum/sumsq over its disjoint half-frame.
  host:     combine partials -> per-channel scale/bias (512 numbers).
  kernel 2: normalize + qkv + attention + proj + residual.

All matmuls run in fp8e4 (TRN e4m3, max +-240) with DoubleRow perf mode:
one instruction contracts TWO 128-deep k-tiles (paired along dim1 of
[128, 2, N] tiles) at 2x bf16 throughput.  Scale management keeps every
fp8 operand in the format's sweet spot (validated on host: rel err vs
reference ~5.7e-3 against a 2e-2 gate):
  - weights are prescaled by WS=16 on the host (else ~27% of N(0,1/512)
    weight entries land in fp8 subnormals); undone by the 1/WS scale on
    the psum->sbuf activation copy.
  - p = exp(score/sqrt(c) - SHIFT), SHIFT=2: max p ~72 < 240, and the
    constant shift cancels exactly in the softmax normalization.
  - attention output is quantized unnormalized as pv/PRE, PRE=WS=16 (max
    |pv| ~530 -> |atB| ~33); because PRE==WS the normalization constant
    is exactly 1/D, applied after the (linear) projection so the PV psum
    banks free up immediately.

Math simplifications used (exact, not approximations):
  - bk drops out of softmax (adds a per-query constant to scores).
  - bv passes through attention unchanged (softmax weights sum to 1), so
    it is folded into the proj bias on the host: bp_eff = bp + wp @ bv.
  - the softmax denominator is the sum of the QUANTIZED p8 (DVE chunk
    adds -> GPSIMD partition all-reduce -> DVE reciprocal; the PE only
    ever executes score/PV/QKV/proj matmuls), so attention weights still
    sum to exactly 1 after normalization.
"""

import numpy as np
import ml_dtypes

import concourse.bass as bass
import concourse.bacc as bacc
import concourse.tile as tile
from concourse import bass_isa, mybir
from concourse.bass_utils import run_bass_kernel_spmd

C = 512
T = 4
HW = 64 * 64          # tokens per frame
HALF = HW // 2        # local queries per core
G = 32                # groups
N_CORES = 8
EPS = 1e-6
NG_ELEMS = (C // G) * T * HW   # elements per group in the full tensor
CB = C // 128         # 4 channel blocks
NP = CB // 2          # 2 channel-block pairs (DoubleRow k-tiles)
QG = HALF // 512      # 4 query groups of 512
NKT = HW // 128       # 32 key chunks of 128
NKP = NKT // 2        # 16 key chunk pairs
SCALE = float(C) ** -0.5
WS = 16.0             # host-side weight prescale (fp8 subnormal dodge)
SHIFT = 2.0           # exp shift: p = exp(s*SCALE - SHIFT), cancels in norm
PRE = 16.0            # attention-out prescale; == WS so bc = exactly 1/D

E4NP = ml_dtypes.float8_e4m3   # TRN fp8e4 semantics (max +-240)

BF16 = mybir.dt.bfloat16
F32 = mybir.dt.float32
FP8 = mybir.dt.float8e4
AX = mybir.AxisListType
AF = mybir.ActivationFunctionType
OP = mybir.AluOpType
DR = mybir.MatmulPerfMode.DoubleRow

_CACHE = {}


# ---------------------------------------------------------------- kernel 1
NCH = 8                   # stat chunks: 2 per channel block (bf16 input)
CHTOK = HALF // 2         # tokens per stat chunk


def _build_stats():
    """Partial sum/sumsq over this core's half-frame (bf16 input halves the
    DMA).  8 chunks of [128, 1024]; sums split DVE/ACT, squares on ACT.
    pstats col c = chunk-c sum, col 8+c = chunk-c sumsq (host combines)."""
    nc = bacc.Bacc("TRN2", target_bir_lowering=False, debug=False,
                   num_devices=N_CORES)
    xh = nc.declare_dram_parameter("xh", [C, HALF], BF16, isOutput=False)
    pstats = nc.declare_dram_parameter("pstats", [128, 2 * NCH], F32,
                                       isOutput=True)
    with tile.TileContext(nc) as tc:
        with tc.tile_pool(name="xt", bufs=NCH) as xt_pool, \
             tc.tile_pool(name="scr", bufs=3) as scr_pool, \
             tc.tile_pool(name="st", bufs=1) as st_pool:
            stats_sb = st_pool.tile([128, 2 * NCH], F32, name="stats")
            for c in range(NCH):
                j, h = divmod(c, 2)
                xt = xt_pool.tile([128, CHTOK], BF16, tag="xt", name="xt")
                eng = nc.sync if c % 2 == 0 else nc.scalar
                eng.dma_start(xt[:, :], xh[j * 128:(j + 1) * 128,
                                           h * CHTOK:(h + 1) * CHTOK])
                if c not in (3, 7):
                    nc.vector.reduce_sum(stats_sb[:, c:c + 1], xt[:, :],
                                         axis=AX.X)
                else:
                    scr = scr_pool.tile([128, CHTOK], F32, tag="scr", name="s0")
                    nc.scalar.activation(scr[:, :], xt[:, :], AF.Identity,
                                         accum_out=stats_sb[:, c:c + 1])
                scr = scr_pool.tile([128, CHTOK], F32, tag="scr", name="scr")
                nc.scalar.activation(scr[:, :], xt[:, :], AF.Square,
                                     accum_out=stats_sb[:, NCH + c:NCH + c + 1])
            nc.sync.dma_start(pstats[:, :], stats_sb[:, :])
    nc.finalize()
    return nc


# ---------------------------------------------------------------- kernel 2
def _body(tc, P):
    from contextlib import ExitStack

    nc = tc.nc
    with ExitStack() as ctx:
        consts = ctx.enter_context(tc.tile_pool(name="consts", bufs=1))

        # scale/bias ride the scalar HWDGE ring first (critical path for
        # the normalize), weights follow; the 8MB xf load rides sync.
        scl_sb = consts.tile([128, CB], F32, name="scl")
        nc.scalar.dma_start(scl_sb[:, :], P["scl2d"][:, :])
        bia_sb = consts.tile([128, CB], F32, name="bia")
        nc.scalar.dma_start(bia_sb[:, :], P["bia2d"][:, :])

        # local half of the frame stays resident: normalize source now,
        # residual read at proj time (saves the 4MB re-read).  DMA'd in
        # 512-token column groups below so the PE starts early.
        xloc = [consts.tile([128, HALF], F32, name=f"xloc{j}")
                for j in range(CB)]

        def wtile(nm):
            t_ = consts.tile([128, 2, 2 * C], FP8, name=nm)
            nc.scalar.dma_start(t_[:, :, :], P[nm][:, :, :])
            return t_

        wq_sb = wtile("wq8")
        wk_sb = wtile("wk8")
        wv_sb = wtile("wv8")
        wp_sb = wtile("wp8")
        bq_sb = consts.tile([128, CB], F32, name="bq")
        nc.scalar.dma_start(bq_sb[:, :], P["bq2d"][:, :])
        bpe_sb = consts.tile([128, CB], F32, name="bpe")
        nc.scalar.dma_start(bpe_sb[:, :], P["bpe2d"][:, :])

        zero_sb = consts.tile([128, 1], F32, name="zero")
        nc.vector.memset(zero_sb[:, :], 0.0)
        nsh_sb = consts.tile([128, 1], F32, name="nsh")
        nc.vector.memset(nsh_sb[:, :], -SHIFT)

        # fp8 activations, channel-block-paired for DoubleRow
        xn_pool = ctx.enter_context(tc.tile_pool(name="xn", bufs=NP))
        xn_sb = [xn_pool.tile([128, 2, HW], FP8, tag="xn", name="xn")
                 for _ in range(NP)]
        q_pool = ctx.enter_context(tc.tile_pool(name="q", bufs=NP))
        q_sb = [q_pool.tile([128, 2, HALF], FP8, tag="q", name="q")
                for _ in range(NP)]
        k_pool = ctx.enter_context(tc.tile_pool(name="k", bufs=NP))
        k_sb = [k_pool.tile([128, 2, HW], FP8, tag="k", name="k")
                for _ in range(NP)]
        v_pool = ctx.enter_context(tc.tile_pool(name="v", bufs=NKP))
        v_sb = [v_pool.tile([128, 2, C], FP8, tag="v", name="v")
                for _ in range(NKP)]

        # psum pools: 4 + 3 + 1 = 8 banks
        ps_mm = ctx.enter_context(tc.tile_pool(name="ps_mm", bufs=4, space="PSUM"))
        ps_st = ctx.enter_context(tc.tile_pool(name="ps_st", bufs=3, space="PSUM"))
        ps_pp = ctx.enter_context(tc.tile_pool(name="ps_pp", bufs=1, space="PSUM"))

        p_pool = ctx.enter_context(tc.tile_pool(name="p", bufs=3))
        acc_pool = ctx.enter_context(tc.tile_pool(name="acc", bufs=2))
        dnr_pool = ctx.enter_context(tc.tile_pool(name="dnr", bufs=2))
        bc_pool = ctx.enter_context(tc.tile_pool(name="bc", bufs=2))
        atB_pool = ctx.enter_context(tc.tile_pool(name="atB", bufs=4))
        ob_pool = ctx.enter_context(tc.tile_pool(name="ob", bufs=4))

        # ---------------- phase 0+1: streamed normalize + q/k/v -------------
        # token-group-major streaming: per 512-token group, DMA + normalize
        # its 4 channel blocks, then immediately emit every matmul that only
        # needs tokens seen so far; the PE starts ~10us earlier than with
        # half-frame-granular loads.
        def norm(j, src, cs):
            nc.vector.tensor_scalar(
                out=xn_sb[j // 2][:, j % 2, cs], in0=src,
                scalar1=scl_sb[:, j:j + 1], scalar2=bia_sb[:, j:j + 1],
                op0=OP.mult, op1=OP.add)

        def qk_group(w_sb, out_sb, j, t_, bias):
            ps = ps_mm.tile([128, 512], F32, tag="mm", name="mm")
            for ip in range(NP):
                nc.tensor.matmul(
                    ps[:, :],
                    lhsT=w_sb[:, :, ip * C + j * 128: ip * C + (j + 1) * 128],
                    rhs=xn_sb[ip][:, :, t_ * 512:(t_ + 1) * 512],
                    start=(ip == 0), stop=(ip == NP - 1), perf_mode=DR)
            dst = out_sb[j // 2][:, j % 2, t_ * 512:(t_ + 1) * 512]
            nc.scalar.activation(dst, ps[:, :], AF.Identity,
                                 scale=1.0 / WS, bias=bias)

        def v_group(m):
            ps = ps_mm.tile([128, 512], F32, tag="mm", name="mm")
            for ip in range(NP):
                nc.tensor.matmul(
                    ps[:, :],
                    lhsT=xn_sb[ip][:, :, m * 128:(m + 1) * 128],
                    rhs=wv_sb[:, :, ip * C:(ip + 1) * C],
                    start=(ip == 0), stop=(ip == NP - 1), perf_mode=DR)
            nc.vector.tensor_scalar(out=v_sb[m // 2][:, m % 2, :], in0=ps[:, :],
                                    scalar1=1.0 / WS, scalar2=None, op0=OP.mult)

        with tc.tile_pool(name="xf", bufs=8) as xf_pool:
            for tg in range(8):
                ts_, te_ = tg * 512, (tg + 1) * 512
                for j in range(CB):
                    if tg < QG:      # local half: land in the resident tiles
                        dst = xloc[j][:, ts_:te_]
                    else:
                        xt = xf_pool.tile([128, 512], F32, tag="xf", name="xf")
                        dst = xt[:, :]
                    nc.sync.dma_start(dst, P["xf"][j * 128:(j + 1) * 128,
                                                   ts_:te_])
                    norm(j, dst, slice(ts_, te_))
                if tg < QG:          # q covers exactly the local half
                    for j in range(CB):
                        qk_group(wq_sb, q_sb, j, tg, bias=bq_sb[:, j:j + 1])
                for j in range(CB):
                    qk_group(wk_sb, k_sb, j, tg, bias=zero_sb[:, :])
                for m in range(4 * tg, 4 * tg + 4):
                    v_group(m)

        # ---------------- phase 2: attention + proj per query group --------
        # proj of group g is emitted at the START of group g+1: its matmuls
        # are ready instantly (own psum bank, inputs done) and fill the PE
        # window where the next score matmuls wait on the exp lag.
        def emit_proj(atB_sb, bc, q0, pool):
            for cb in range(CB):
                pp = pool.tile([128, 512], F32, tag="mm" if pool is ps_mm
                               else "pp", name="pp")
                for ip in range(NP):
                    nc.tensor.matmul(
                        pp[:, :],
                        lhsT=wp_sb[:, :, ip * C + cb * 128: ip * C + (cb + 1) * 128],
                        rhs=atB_sb[ip][:, :, :],
                        start=(ip == 0), stop=(ip == NP - 1), perf_mode=DR)
                t1 = ob_pool.tile([128, 512], F32, tag="t1", name="t1")
                nc.vector.tensor_mul(t1[:, :], pp[:, :], bc[:, :])
                ob = ob_pool.tile([128, 512], F32, tag="ob", name="ob")
                nc.vector.scalar_tensor_tensor(
                    ob[:, :], in0=t1[:, :], scalar=bpe_sb[:, cb:cb + 1],
                    in1=xloc[cb][:, q0:q0 + 512], op0=OP.add, op1=OP.add)
                nc.sync.dma_start(P["out"][cb * 128:(cb + 1) * 128, q0:q0 + 512],
                                  ob[:, :])

        def finalize_group(atB_sb, acc, q0, pool):
            # denominator -> bc = 1/D, entirely off the PE: GPSIMD
            # all-reduces the DVE partials across partitions (result lands
            # replicated on all 128), DVE takes the exact f32 reciprocal.
            dall = dnr_pool.tile([128, 512], F32, tag="dnr", name="dall")
            nc.gpsimd.partition_all_reduce(dall[:, :], acc[:, :], 128,
                                           bass_isa.ReduceOp.add)
            bc = bc_pool.tile([128, 512], F32, tag="bc", name="bc")
            nc.vector.reciprocal(bc[:, :], dall[:, :])
            emit_proj(atB_sb, bc, q0, pool)

        deferred = None
        for qg in range(QG):
            q0 = qg * 512
            pv = [ps_mm.tile([128, 512], F32, tag="mm", name="mm")
                  for _ in range(CB)]
            acc = acc_pool.tile([128, 512], F32, tag="acc", name="acc")

            def pvmm(m2_, p8_, start, stop):
                for cb in range(CB):
                    # attention output channel-major: out[co, qt] += v^T p
                    nc.tensor.matmul(
                        pv[cb][:, :],
                        lhsT=v_sb[m2_][:, :, cb * 128:(cb + 1) * 128],
                        rhs=p8_[:, :, :],
                        start=start, stop=stop, perf_mode=DR)

            # software-pipelined by one pair: PV of pair m2-1 is emitted
            # after the scores of pair m2, so the PE never waits on exp.
            p8s = []
            for m2 in range(NKP):
                p8 = p_pool.tile([128, 2, 512], FP8, tag="p", name="p")
                for mm in range(2):
                    m = 2 * m2 + mm
                    st = ps_st.tile([128, 512], F32, tag="st", name="st")
                    for ip in range(NP):
                        nc.tensor.matmul(
                            st[:, :],
                            lhsT=k_sb[ip][:, :, m * 128:(m + 1) * 128],
                            rhs=q_sb[ip][:, :, q0:q0 + 512],
                            start=(ip == 0), stop=(ip == NP - 1), perf_mode=DR)
                    nc.scalar.activation(p8[:, mm, :], st[:, :], AF.Exp,
                                         scale=SCALE, bias=nsh_sb[:, :])
                if m2 == 1 and deferred is not None:
                    # previous group's denominator+proj, emitted after two
                    # pairs of scores so the PE stays fed while the
                    # GPSIMD/DVE chain finishes
                    finalize_group(*deferred, pool=ps_pp)
                    deferred = None
                # denominator partials ride the DVE (the PE only ever sees
                # score/PV/proj matmuls)
                if m2 == 0:
                    nc.vector.tensor_add(acc[:, :], p8[:, 0, :], p8[:, 1, :])
                else:
                    nc.vector.tensor_add(acc[:, :], acc[:, :], p8[:, 0, :])
                    nc.vector.tensor_add(acc[:, :], acc[:, :], p8[:, 1, :])
                if m2 > 0:
                    pvmm(m2 - 1, p8s[m2 - 1], start=(m2 == 1), stop=False)
                p8s.append(p8)
            pvmm(NKP - 1, p8s[NKP - 1], start=False, stop=True)
            # quantize UNNORMALIZED attention out of PSUM right away (frees
            # the pv banks for the next query group); the denominator is
            # applied after the (linear) projection instead.
            atB_sb = []
            for pr in range(NP):
                atB = atB_pool.tile([128, 2, 512], FP8, tag="atB", name="atB")
                for i in range(2):
                    nc.scalar.activation(atB[:, i, :], pv[2 * pr + i][:, :],
                                         AF.Identity, scale=1.0 / PRE,
                                         bias=zero_sb[:, :])
                atB_sb.append(atB)
            deferred = (atB_sb, acc, q0)
        finalize_group(*deferred, pool=ps_mm)


def _build_main():
    nc = bacc.Bacc("TRN2", target_bir_lowering=False, debug=False,
                   num_devices=N_CORES)
    P = {}
    P["xf"] = nc.declare_dram_parameter("xf", [C, HW], F32, isOutput=False)
    for nm in ("wq8", "wk8", "wv8", "wp8"):
        P[nm] = nc.declare_dram_parameter(nm, [128, 2, 2 * C], FP8,
                                          isOutput=False)
    for nm in ("bq2d", "bpe2d", "scl2d", "bia2d"):
        P[nm] = nc.declare_dram_parameter(nm, [128, CB], F32, isOutput=False)
    P["out"] = nc.declare_dram_parameter("out", [C, HALF], F32, isOutput=True)

    with tile.TileContext(nc) as tc:
        _body(tc, P)
    nc.finalize()
    return nc


def _get_ncs():
    if "nc" not in _CACHE:
        _CACHE["nc1"] = _build_stats()
        _CACHE["nc"] = _build_main()
    return _CACHE["nc1"], _CACHE["nc"]


def _frame_views(x):
    """Per-core rolled frame views: core i=(2f+h) sees frame f with its own
    half first."""
    views = []
    for i in range(N_CORES):
        f, h = divmod(i, 2)
        xfr = x[0, :, f].reshape(C, HW)
        if h == 1:
            xfr = np.concatenate([xfr[:, HALF:], xfr[:, :HALF]], axis=1)
        views.append(np.ascontiguousarray(xfr))
    return views


def _combine_stats(pstats_list, gamma, beta):
    """Host-side gather of kernel-1 partials -> per-channel scale/bias."""
    tot = np.zeros((128, 2 * NCH), np.float64)
    for ps in pstats_list:
        tot += np.asarray(ps, np.float64)
    # chunk c covers channel block c//2: fold the two token-halves
    sc = tot[:, 0:NCH].reshape(128, CB, 2).sum(2)       # [128, CB] sums
    sc2 = tot[:, NCH:2 * NCH].reshape(128, CB, 2).sum(2)
    s = sc.T.reshape(C)       # per-channel sum
    s2 = sc2.T.reshape(C)     # per-channel sumsq
    gs = s.reshape(G, C // G).sum(1)
    gs2 = s2.reshape(G, C // G).sum(1)
    meang = gs / NG_ELEMS
    varg = gs2 / NG_ELEMS - meang * meang
    rstd = 1.0 / np.sqrt(varg + EPS)
    chs = (np.asarray(gamma, np.float64) * np.repeat(rstd, C // G))
    chb = np.asarray(beta, np.float64) - np.repeat(meang, C // G) * chs
    def blk2d(v):
        return np.ascontiguousarray(v.astype(np.float32).reshape(CB, 128).T)
    return blk2d(chs), blk2d(chb)


def _w8pack(w):
    """(c_out, c_in) f32 -> [128, 2, 2C] fp8e4, channel-block-pair packed:
    out[p, i, ip*C + o] = w.T[(2*ip + i)*128 + p, o] * WS."""
    a = (np.asarray(w, np.float32).T * WS).reshape(2, 2, 128, C)
    a = np.ascontiguousarray(a.transpose(2, 1, 0, 3).reshape(128, 2, 2 * C))
    return a.astype(E4NP)


def run_with_results(inputs, trace=False, **kw):
    f32 = np.float32
    x = np.asarray(inputs["x"], f32)
    gamma = np.asarray(inputs["gamma"], f32)
    beta = np.asarray(inputs["beta"], f32)
    wq, wk, wv, wp = [np.asarray(inputs[n], f32) for n in ("wq", "wk", "wv", "wp")]
    bq, bv, bp = [np.asarray(inputs[n], f32) for n in ("bq", "bv", "bp")]

    nc1, nc2 = _get_ncs()
    views = _frame_views(x)

    # ---- launch 1: partial GroupNorm stats over disjoint half-frames
    # (bf16 input: halves the DMA; the stats shift is far below the gate)
    maps1 = [{"xh": np.ascontiguousarray(views[i][:, :HALF])
              .astype(ml_dtypes.bfloat16)}
             for i in range(N_CORES)]
    res1 = run_bass_kernel_spmd(nc1, maps1, core_ids=list(range(N_CORES)),
                                trace=trace, **kw)
    scl2d, bia2d = _combine_stats([r["pstats"] for r in res1.results],
                                  gamma, beta)

    # ---- launch 2: the block itself
    def blk2d(v):
        return np.ascontiguousarray(np.asarray(v, f32).reshape(CB, 128).T)

    shared = {
        "wq8": _w8pack(wq), "wk8": _w8pack(wk), "wv8": _w8pack(wv),
        "wp8": _w8pack(wp),
        "bq2d": blk2d(bq), "bpe2d": blk2d(bp + wp @ bv),
        "scl2d": scl2d, "bia2d": bia2d,
    }
    maps2 = [dict(shared, xf=views[i]) for i in range(N_CORES)]
    res2 = run_bass_kernel_spmd(nc2, maps2, core_ids=list(range(N_CORES)),
                                trace=trace, **kw)

    frames = []
    for f in range(T):
        a = np.asarray(res2.results[2 * f]["out"], dtype=np.float32)
        b = np.asarray(res2.results[2 * f + 1]["out"], dtype=np.float32)
        frames.append(np.concatenate([a, b], axis=1))
    out = np.stack(frames, axis=1)           # (C, T, HW)
    out = np.ascontiguousarray(out.reshape(1, C, T, 64, 64))
    return out, (res1, res2)


def kernel(**inputs):
    out, _ = run_with_results(inputs)
    return out


# revision 30
# speedup vs baseline: 1.7409x; 1.0185x over previous
"""GroupNorm + per-frame spatial attention block on 8 TRN2 NeuronCores.

Problem shape: x (1, 512, 4, 64, 64) f32.
  y   = GroupNorm32(x) (stats over (c/32, t, h, w) -> global over all frames)
  tok = y as (t, hw=4096, c=512)
  q,k,v = tok @ w{q,k,v}.T + b ; per-frame softmax(q k^T / sqrt(c)) v
  out = attn @ wp.T + bp ; return x + out

Sharding: core i handles frame f=i//2, query-half h=i%2 (2048 queries).
Each core redundantly computes K/V for its whole frame (cheaper than an
intra-pair all-gather).

Two launches (a fleet-wide collective barrier costs ~65us of latency, so
the tiny GroupNorm stats reduction is done as its own collective-free
kernel; the host combines the 8x[128,8] partial sums while "gathering"):
  kernel 1: per-core partial sum/sumsq over its disjoint half-frame.
  host:     combine partials -> per-channel scale/bias (512 numbers).
  kernel 2: normalize + qkv + attention + proj + residual.

All matmuls run in fp8e4 (TRN e4m3, max +-240) with DoubleRow perf mode:
one instruction contracts TWO 128-deep k-tiles (paired along dim1 of
[128, 2, N] tiles) at 2x bf16 throughput.  Scale management keeps every
fp8 operand in the format's sweet spot (validated on host: rel err vs
reference ~5.7e-3 against a 2e-2 gate):
  - weights are prescaled by WS=16 on the host (else ~27% of N(0,1/512)
    weight entries land in fp8 subnormals); undone by the 1/WS scale on
    the psum->sbuf activation copy.
  - p = exp(score/sqrt(c) - SHIFT), SHIFT=2: max p ~72 < 240, and the
    constant shift cancels exactly in the softmax normalization.
  - attention output is quantized unnormalized as pv/PRE, PRE=WS=16 (max
    |pv| ~530 -> |atB| ~33); because PRE==WS the normalization constant
    is exactly 1/D, applied after the (linear) projection so the PV psum
    banks free up immediately.

Math simplifications used (exact, not approximations):
  - bk drops out of softmax (adds a per-query constant to scores).
  - bv passes through attention unchanged (softmax weights sum to 1), so
    it is folded into the proj bias on the host: bp_eff = bp + wp @ bv.
  - the softmax denominator is the sum of the QUANTIZED p8 (DVE chunk
    adds -> GPSIMD partition all-reduce -> DVE reciprocal; the PE only
    ever executes score/PV/QKV/proj matmuls), so attention weights still
    sum to exactly 1 after normalization.
"""

import numpy as np
import ml_dtypes

import concourse.bass as bass
import concourse.bacc as bacc
import concourse.tile as tile
from concourse import bass_isa, mybir
from concourse.bass_utils import run_bass_kernel_spmd

C = 512
T = 4
HW = 64 * 64          # tokens per frame
HALF = HW // 2        # local queries per core
G = 32                # groups
N_CORES = 8
EPS = 1e-6
NG_ELEMS = (C // G) * T * HW   # elements per group in the full tensor
CB = C // 128         # 4 channel blocks
NP = CB // 2          # 2 channel-block pairs (DoubleRow k-tiles)
QG = HALF // 512      # 4 query groups of 512
NKT = HW // 128       # 32 key chunks of 128
NKP = NKT // 2        # 16 key chunk pairs
SCALE = float(C) ** -0.5
WS = 16.0             # host-side weight prescale (fp8 subnormal dodge)
SHIFT = 2.0           # exp shift: p = exp(s*SCALE - SHIFT), cancels in norm
PRE = 16.0            # attention-out prescale; == WS so bc = exactly 1/D

E4NP = ml_dtypes.float8_e4m3   # TRN fp8e4 semantics (max +-240)

BF16 = mybir.dt.bfloat16
F32 = mybir.dt.float32
FP8 = mybir.dt.float8e4
AX = mybir.AxisListType
AF = mybir.ActivationFunctionType
OP = mybir.AluOpType
DR = mybir.MatmulPerfMode.DoubleRow

_CACHE = {}


# ---------------------------------------------------------------- kernel 1
def _build_stats():
    """Partial sum/sumsq over this core's half-frame.  bf16 input halves
    the DMA; each channel-block tile is DMA'd in two halves on the two
    rings; big [128, 2048] ops amortize the per-op engine overhead (sums
    on DVE, squares+accum on ACT, running in parallel)."""
    nc = bacc.Bacc("TRN2", target_bir_lowering=False, debug=False,
                   num_devices=N_CORES)
    xh = nc.declare_dram_parameter("xh", [C, HALF], BF16, isOutput=False)
    pstats = nc.declare_dram_parameter("pstats", [128, 2 * CB], F32,
                                       isOutput=True)
    with tile.TileContext(nc) as tc:
        with tc.tile_pool(name="xt", bufs=CB) as xt_pool, \
             tc.tile_pool(name="scr", bufs=2) as scr_pool, \
             tc.tile_pool(name="st", bufs=1) as st_pool:
            stats_sb = st_pool.tile([128, 2 * CB], F32, name="stats")
            for j in range(CB):
                xt = xt_pool.tile([128, HALF], BF16, tag="xt", name="xt")
                r = xh[j * 128:(j + 1) * 128, :]
                nc.sync.dma_start(xt[:, 0:HALF // 2], r[:, 0:HALF // 2])
                nc.scalar.dma_start(xt[:, HALF // 2:HALF], r[:, HALF // 2:HALF])
                nc.vector.reduce_sum(stats_sb[:, j:j + 1], xt[:, :], axis=AX.X)
                scr = scr_pool.tile([128, HALF], F32, tag="scr", name="scr")
                nc.scalar.activation(scr[:, :], xt[:, :], AF.Square,
                                     accum_out=stats_sb[:, CB + j:CB + j + 1])
            nc.sync.dma_start(pstats[:, :], stats_sb[:, :])
    nc.finalize()
    return nc


# ---------------------------------------------------------------- kernel 2
def _body(tc, P):
    from contextlib import ExitStack

    nc = tc.nc
    with ExitStack() as ctx:
        consts = ctx.enter_context(tc.tile_pool(name="consts", bufs=1))

        # scale/bias ride the scalar HWDGE ring first (critical path for
        # the normalize), weights follow; the 8MB xf load rides sync.
        scl_sb = consts.tile([128, CB], F32, name="scl")
        nc.scalar.dma_start(scl_sb[:, :], P["scl2d"][:, :])
        bia_sb = consts.tile([128, CB], F32, name="bia")
        nc.scalar.dma_start(bia_sb[:, :], P["bia2d"][:, :])

        # local half of the frame stays resident: normalize source now,
        # residual read at proj time (saves the 4MB re-read).  DMA'd in
        # 512-token column groups below so the PE starts early.
        xloc = [consts.tile([128, HALF], F32, name=f"xloc{j}")
                for j in range(CB)]

        def wtile(nm):
            t_ = consts.tile([128, 2, 2 * C], FP8, name=nm)
            nc.scalar.dma_start(t_[:, :, :], P[nm][:, :, :])
            return t_

        wq_sb = wtile("wq8")
        wk_sb = wtile("wk8")
        wv_sb = wtile("wv8")
        wp_sb = wtile("wp8")
        bq_sb = consts.tile([128, CB], F32, name="bq")
        nc.scalar.dma_start(bq_sb[:, :], P["bq2d"][:, :])
        bpe_sb = consts.tile([128, CB], F32, name="bpe")
        nc.scalar.dma_start(bpe_sb[:, :], P["bpe2d"][:, :])

        onesf_sb = consts.tile([128, 1], F32, name="onesf")
        nc.vector.memset(onesf_sb[:, :], 1.0)
        onesrow_sb = consts.tile([1, 128], BF16, name="onesrow")
        nc.vector.memset(onesrow_sb[:, :], 1.0)
        zero_sb = consts.tile([128, 1], F32, name="zero")
        nc.vector.memset(zero_sb[:, :], 0.0)
        nsh_sb = consts.tile([128, 1], F32, name="nsh")
        nc.vector.memset(nsh_sb[:, :], -SHIFT)

        # fp8 activations, channel-block-paired for DoubleRow
        xn_pool = ctx.enter_context(tc.tile_pool(name="xn", bufs=NP))
        xn_sb = [xn_pool.tile([128, 2, HW], FP8, tag="xn", name="xn")
                 for _ in range(NP)]
        q_pool = ctx.enter_context(tc.tile_pool(name="q", bufs=NP))
        q_sb = [q_pool.tile([128, 2, HALF], FP8, tag="q", name="q")
                for _ in range(NP)]
        k_pool = ctx.enter_context(tc.tile_pool(name="k", bufs=NP))
        k_sb = [k_pool.tile([128, 2, HW], FP8, tag="k", name="k")
                for _ in range(NP)]
        v_pool = ctx.enter_context(tc.tile_pool(name="v", bufs=NKP))
        v_sb = [v_pool.tile([128, 2, C], FP8, tag="v", name="v")
                for _ in range(NKP)]

        # psum pools: 4 + 2 + 2 = 8 banks
        ps_mm = ctx.enter_context(tc.tile_pool(name="ps_mm", bufs=4, space="PSUM"))
        ps_st = ctx.enter_context(tc.tile_pool(name="ps_st", bufs=2, space="PSUM"))
        ps_pp = ctx.enter_context(tc.tile_pool(name="ps_pp", bufs=2, space="PSUM"))

        p_pool = ctx.enter_context(tc.tile_pool(name="p", bufs=4))
        acc_pool = ctx.enter_context(tc.tile_pool(name="acc", bufs=2))
        dnr_pool = ctx.enter_context(tc.tile_pool(name="dnr", bufs=2))
        bc_pool = ctx.enter_context(tc.tile_pool(name="bc", bufs=2))
        atB_pool = ctx.enter_context(tc.tile_pool(name="atB", bufs=4))
        ob_pool = ctx.enter_context(tc.tile_pool(name="ob", bufs=4))

        # ---------------- phase 0+1: streamed normalize + q/k/v -------------
        # token-group-major streaming: per 512-token group, DMA + normalize
        # its 4 channel blocks, then immediately emit every matmul that only
        # needs tokens seen so far; the PE starts ~10us earlier than with
        # half-frame-granular loads.
        def norm(j, src, cs):
            nc.vector.tensor_scalar(
                out=xn_sb[j // 2][:, j % 2, cs], in0=src,
                scalar1=scl_sb[:, j:j + 1], scalar2=bia_sb[:, j:j + 1],
                op0=OP.mult, op1=OP.add)

        def qk_group(w_sb, out_sb, j, t_, bias):
            ps = ps_mm.tile([128, 512], F32, tag="mm", name="mm")
            for ip in range(NP):
                nc.tensor.matmul(
                    ps[:, :],
                    lhsT=w_sb[:, :, ip * C + j * 128: ip * C + (j + 1) * 128],
                    rhs=xn_sb[ip][:, :, t_ * 512:(t_ + 1) * 512],
                    start=(ip == 0), stop=(ip == NP - 1), perf_mode=DR)
            dst = out_sb[j // 2][:, j % 2, t_ * 512:(t_ + 1) * 512]
            nc.scalar.activation(dst, ps[:, :], AF.Identity,
                                 scale=1.0 / WS, bias=bias)

        def v_group(m):
            ps = ps_mm.tile([128, 512], F32, tag="mm", name="mm")
            for ip in range(NP):
                nc.tensor.matmul(
                    ps[:, :],
                    lhsT=xn_sb[ip][:, :, m * 128:(m + 1) * 128],
                    rhs=wv_sb[:, :, ip * C:(ip + 1) * C],
                    start=(ip == 0), stop=(ip == NP - 1), perf_mode=DR)
            nc.vector.tensor_scalar(out=v_sb[m // 2][:, m % 2, :], in0=ps[:, :],
                                    scalar1=1.0 / WS, scalar2=None, op0=OP.mult)

        with tc.tile_pool(name="xf", bufs=8) as xf_pool:
            for tg in range(8):
                ts_, te_ = tg * 512, (tg + 1) * 512
                for j in range(CB):
                    if tg < QG:      # local half: land in the resident tiles
                        dst = xloc[j][:, ts_:te_]
                    else:
                        xt = xf_pool.tile([128, 512], F32, tag="xf", name="xf")
                        dst = xt[:, :]
                    nc.sync.dma_start(dst, P["xf"][j * 128:(j + 1) * 128,
                                                   ts_:te_])
                    norm(j, dst, slice(ts_, te_))
                if tg < QG:          # q covers exactly the local half
                    for j in range(CB):
                        qk_group(wq_sb, q_sb, j, tg, bias=bq_sb[:, j:j + 1])
                for j in range(CB):
                    qk_group(wk_sb, k_sb, j, tg, bias=zero_sb[:, :])
                for m in range(4 * tg, 4 * tg + 4):
                    v_group(m)

        # ---------------- phase 2: attention + proj per query group --------
        # proj of group g is emitted at the START of group g+1: its matmuls
        # are ready instantly (own psum bank, inputs done) and fill the PE
        # window where the next score matmuls wait on the exp lag.
        def emit_proj(atB_sb, bc, q0, pool):
            for cb in range(CB):
                pp = pool.tile([128, 512], F32, tag="mm" if pool is ps_mm
                               else "pp", name="pp")
                for ip in range(NP):
                    nc.tensor.matmul(
                        pp[:, :],
                        lhsT=wp_sb[:, :, ip * C + cb * 128: ip * C + (cb + 1) * 128],
                        rhs=atB_sb[ip][:, :, :],
                        start=(ip == 0), stop=(ip == NP - 1), perf_mode=DR)
                t1 = ob_pool.tile([128, 512], F32, tag="t1", name="t1")
                nc.vector.tensor_mul(t1[:, :], pp[:, :], bc[:, :])
                ob = ob_pool.tile([128, 512], F32, tag="ob", name="ob")
                nc.vector.scalar_tensor_tensor(
                    ob[:, :], in0=t1[:, :], scalar=bpe_sb[:, cb:cb + 1],
                    in1=xloc[cb][:, q0:q0 + 512], op0=OP.add, op1=OP.add)
                nc.sync.dma_start(P["out"][cb * 128:(cb + 1) * 128, q0:q0 + 512],
                                  ob[:, :])

        def finalize_group(atB_sb, acc, q0, pool):
            # denominator -> bc = 1/D: ones-matmul partition-reduce of the
            # DVE partials, reciprocal (bf16: ~0.4% on 1/D, way below the
            # gate), rank-1 broadcast back to 128 partitions.  ~1.1us of
            # PE per group, with every wait already satisfied when emitted.
            dnr = ps_pp.tile([1, 512], F32, tag="pp", name="dnr")
            nc.tensor.matmul(dnr[:, :], lhsT=onesf_sb[:, :], rhs=acc[:, :],
                             start=True, stop=True)
            dnrec = dnr_pool.tile([1, 512], BF16, tag="dnr", name="dnrec")
            with nc.allow_low_precision("bf16 1/denominator on a 2e-2 gate"):
                nc.vector.reciprocal(dnrec[:, :], dnr[:, :])
            bcp = ps_pp.tile([128, 512], F32, tag="pp", name="bcp")
            nc.tensor.matmul(bcp[:, :], lhsT=onesrow_sb[:, :], rhs=dnrec[:, :],
                             start=True, stop=True)
            bc = bc_pool.tile([128, 512], F32, tag="bc", name="bc")
            nc.vector.tensor_copy(bc[:, :], bcp[:, :])
            emit_proj(atB_sb, bc, q0, pool)

        deferred = None
        for qg in range(QG):
            q0 = qg * 512
            pv = [ps_mm.tile([128, 512], F32, tag="mm", name="mm")
                  for _ in range(CB)]
            acc = acc_pool.tile([128, 512], F32, tag="acc", name="acc")

            def pvmm(m2_, p8_, start, stop):
                for cb in range(CB):
                    # attention output channel-major: out[co, qt] += v^T p
                    nc.tensor.matmul(
                        pv[cb][:, :],
                        lhsT=v_sb[m2_][:, :, cb * 128:(cb + 1) * 128],
                        rhs=p8_[:, :, :],
                        start=start, stop=stop, perf_mode=DR)

            def acc_adds(m2_, p8_):
                # denominator partials ride the DVE (the PE only ever sees
                # score/PV/proj matmuls)
                if m2_ == 0:
                    nc.vector.tensor_add(acc[:, :], p8_[:, 0, :], p8_[:, 1, :])
                else:
                    nc.vector.tensor_add(acc[:, :], acc[:, :], p8_[:, 0, :])
                    nc.vector.tensor_add(acc[:, :], acc[:, :], p8_[:, 1, :])

            # software-pipelined by two pairs: PV of pair m2-2 is emitted
            # after the scores of pair m2, so the PE has ~3us of runway at
            # a group boundary before anything depends on the previous
            # group's atB quantization or denominator chain.
            p8s = []
            for m2 in range(NKP):
                p8 = p_pool.tile([128, 2, 512], FP8, tag="p", name="p")
                for mm in range(2):
                    m = 2 * m2 + mm
                    st = ps_st.tile([128, 512], F32, tag="st", name="st")
                    for ip in range(NP):
                        nc.tensor.matmul(
                            st[:, :],
                            lhsT=k_sb[ip][:, :, m * 128:(m + 1) * 128],
                            rhs=q_sb[ip][:, :, q0:q0 + 512],
                            start=(ip == 0), stop=(ip == NP - 1), perf_mode=DR)
                    nc.scalar.activation(p8[:, mm, :], st[:, :], AF.Exp,
                                         scale=SCALE, bias=nsh_sb[:, :])
                if m2 == 1:
                    if deferred is not None:
                        finalize_group(*deferred, pool=ps_pp)
                        deferred = None
                    acc_adds(0, p8s[0])
                if m2 >= 1:
                    acc_adds(m2, p8)
                if m2 >= 2:
                    pvmm(m2 - 2, p8s[m2 - 2], start=(m2 == 2), stop=False)
                p8s.append(p8)
            pvmm(NKP - 2, p8s[NKP - 2], start=False, stop=False)
            pvmm(NKP - 1, p8s[NKP - 1], start=False, stop=True)
            # quantize UNNORMALIZED attention out of PSUM right away (frees
            # the pv banks for the next query group); the denominator is
            # applied after the (linear) projection instead.  Split
            # ACT/DVE so the copies land in ~1.3us instead of 2.3.
            atB_sb = []
            for pr in range(NP):
                atB = atB_pool.tile([128, 2, 512], FP8, tag="atB", name="atB")
                nc.scalar.activation(atB[:, 0, :], pv[2 * pr][:, :],
                                     AF.Identity, scale=1.0 / PRE,
                                     bias=zero_sb[:, :])
                nc.vector.tensor_scalar(out=atB[:, 1, :],
                                        in0=pv[2 * pr + 1][:, :],
                                        scalar1=1.0 / PRE, scalar2=None,
                                        op0=OP.mult)
                atB_sb.append(atB)
            deferred = (atB_sb, acc, q0)
        finalize_group(*deferred, pool=ps_mm)


def _build_main():
    nc = bacc.Bacc("TRN2", target_bir_lowering=False, debug=False,
                   num_devices=N_CORES)
    P = {}
    P["xf"] = nc.declare_dram_parameter("xf", [C, HW], F32, isOutput=False)
    for nm in ("wq8", "wk8", "wv8", "wp8"):
        P[nm] = nc.declare_dram_parameter(nm, [128, 2, 2 * C], FP8,
                                          isOutput=False)
    for nm in ("bq2d", "bpe2d", "scl2d", "bia2d"):
        P[nm] = nc.declare_dram_parameter(nm, [128, CB], F32, isOutput=False)
    P["out"] = nc.declare_dram_parameter("out", [C, HALF], F32, isOutput=True)

    with tile.TileContext(nc) as tc:
        _body(tc, P)
    nc.finalize()
    return nc


def _get_ncs():
    if "nc" not in _CACHE:
        _CACHE["nc1"] = _build_stats()
        _CACHE["nc"] = _build_main()
    return _CACHE["nc1"], _CACHE["nc"]


def _frame_views(x):
    """Per-core rolled frame views: core i=(2f+h) sees frame f with its own
    half first."""
    views = []
    for i in range(N_CORES):
        f, h = divmod(i, 2)
        xfr = x[0, :, f].reshape(C, HW)
        if h == 1:
            xfr = np.concatenate([xfr[:, HALF:], xfr[:, :HALF]], axis=1)
        views.append(np.ascontiguousarray(xfr))
    return views


def _combine_stats(pstats_list, gamma, beta):
    """Host-side gather of kernel-1 partials -> per-channel scale/bias."""
    tot = np.zeros((128, 2 * NCH), np.float64)
    for ps in pstats_list:
        tot += np.asarray(ps, np.float64)
    # chunk c covers channel block c//2: fold the two token-halves
    sc = tot[:, 0:NCH].reshape(128, CB, 2).sum(2)       # [128, CB] sums
    sc2 = tot[:, NCH:2 * NCH].reshape(128, CB, 2).sum(2)
    s = sc.T.reshape(C)       # per-channel sum
    s2 = sc2.T.reshape(C)     # per-channel sumsq
    gs = s.reshape(G, C // G).sum(1)
    gs2 = s2.reshape(G, C // G).sum(1)
    meang = gs / NG_ELEMS
    varg = gs2 / NG_ELEMS - meang * meang
    rstd = 1.0 / np.sqrt(varg + EPS)
    chs = (np.asarray(gamma, np.float64) * np.repeat(rstd, C // G))
    chb = np.asarray(beta, np.float64) - np.repeat(meang, C // G) * chs
    def blk2d(v):
        return np.ascontiguousarray(v.astype(np.float32).reshape(CB, 128).T)
    return blk2d(chs), blk2d(chb)


def _w8pack(w):
    """(c_out, c_in) f32 -> [128, 2, 2C] fp8e4, channel-block-pair packed:
    out[p, i, ip*C + o] = w.T[(2*ip + i)*128 + p, o] * WS."""
    a = (np.asarray(w, np.float32).T * WS).reshape(2, 2, 128, C)
    a = np.ascontiguousarray(a.transpose(2, 1, 0, 3).reshape(128, 2, 2 * C))
    return a.astype(E4NP)


def run_with_results(inputs, trace=False, **kw):
    f32 = np.float32
    x = np.asarray(inputs["x"], f32)
    gamma = np.asarray(inputs["gamma"], f32)
    beta = np.asarray(inputs["beta"], f32)
    wq, wk, wv, wp = [np.asarray(inputs[n], f32) for n in ("wq", "wk", "wv", "wp")]
    bq, bv, bp = [np.asarray(inputs[n], f32) for n in ("bq", "bv", "bp")]

    nc1, nc2 = _get_ncs()
    views = _frame_views(x)

    # ---- launch 1: partial GroupNorm stats over disjoint half-frames
    # (bf16 input: halves the DMA; the stats shift is far below the gate)
    maps1 = [{"xh": np.ascontiguousarray(views[i][:, :HALF])
              .astype(ml_dtypes.bfloat16)}
             for i in range(N_CORES)]
    res1 = run_bass_kernel_spmd(nc1, maps1, core_ids=list(range(N_CORES)),
                                trace=trace, **kw)
    scl2d, bia2d = _combine_stats([r["pstats"] for r in res1.results],
                                  gamma, beta)

    # ---- launch 2: the block itself
    def blk2d(v):
        return np.ascontiguousarray(np.asarray(v, f32).reshape(CB, 128).T)

    shared = {
        "wq8": _w8pack(wq), "wk8": _w8pack(wk), "wv8": _w8pack(wv),
        "wp8": _w8pack(wp),
        "bq2d": blk2d(bq), "bpe2d": blk2d(bp + wp @ bv),
        "scl2d": scl2d, "bia2d": bia2d,
    }
    maps2 = [dict(shared, xf=views[i]) for i in range(N_CORES)]
    res2 = run_bass_kernel_spmd(nc2, maps2, core_ids=list(range(N_CORES)),
                                trace=trace, **kw)

    frames = []
    for f in range(T):
        a = np.asarray(res2.results[2 * f]["out"], dtype=np.float32)
        b = np.asarray(res2.results[2 * f + 1]["out"], dtype=np.float32)
        frames.append(np.concatenate([a, b], axis=1))
    out = np.stack(frames, axis=1)           # (C, T, HW)
    out = np.ascontiguousarray(out.reshape(1, C, T, 64, 64))
    return out, (res1, res2)


def kernel(**inputs):
    out, _ = run_with_results(inputs)
    return out


# revision 34
# speedup vs baseline: 1.7434x; 1.0014x over previous
"""GroupNorm + per-frame spatial attention block on 8 TRN2 NeuronCores.

Problem shape: x (1, 512, 4, 64, 64) f32.
  y   = GroupNorm32(x) (stats over (c/32, t, h, w) -> global over all frames)
  tok = y as (t, hw=4096, c=512)
  q,k,v = tok @ w{q,k,v}.T + b ; per-frame softmax(q k^T / sqrt(c)) v
  out = attn @ wp.T + bp ; return x + out

Sharding: core i handles frame f=i//2, query-half h=i%2 (2048 queries).
Each core redundantly computes K/V for its whole frame (cheaper than an
intra-pair all-gather).

Two launches (a fleet-wide collective barrier costs ~65us of latency, so
the tiny GroupNorm stats reduction is done as its own collective-free
kernel; the host combines the 8x[128,8] partial sums while "gathering"):
  kernel 1: per-core partial sum/sumsq over its disjoint half-frame.
  host:     combine partials -> per-channel scale/bias (512 numbers).
  kernel 2: normalize + qkv + attention + proj + residual.

All matmuls run in fp8e4 (TRN e4m3, max +-240) with DoubleRow perf mode:
one instruction contracts TWO 128-deep k-tiles (paired along dim1 of
[128, 2, N] tiles) at 2x bf16 throughput.  Scale management keeps every
fp8 operand in the format's sweet spot (validated on host: rel err vs
reference ~5.7e-3 against a 2e-2 gate):
  - weights are prescaled by WS=16 on the host (else ~27% of N(0,1/512)
    weight entries land in fp8 subnormals); undone by the 1/WS scale on
    the psum->sbuf activation copy.
  - p = exp(score/sqrt(c) - SHIFT), SHIFT=2: max p ~72 < 240, and the
    constant shift cancels exactly in the softmax normalization.
  - attention output is quantized unnormalized as pv/PRE, PRE=WS=16 (max
    |pv| ~530 -> |atB| ~33); because PRE==WS the normalization constant
    is exactly 1/D, applied after the (linear) projection so the PV psum
    banks free up immediately.

Math simplifications used (exact, not approximations):
  - bk drops out of softmax (adds a per-query constant to scores).
  - bv passes through attention unchanged (softmax weights sum to 1), so
    it is folded into the proj bias on the host: bp_eff = bp + wp @ bv.
  - the softmax denominator is the sum of the QUANTIZED p8 (DVE chunk
    adds -> GPSIMD partition all-reduce -> DVE reciprocal; the PE only
    ever executes score/PV/QKV/proj matmuls), so attention weights still
    sum to exactly 1 after normalization.
"""

import numpy as np
import ml_dtypes

import concourse.bass as bass
import concourse.bacc as bacc
import concourse.tile as tile
from concourse import bass_isa, mybir
from concourse.bass_utils import run_bass_kernel_spmd

C = 512
T = 4
HW = 64 * 64          # tokens per frame
HALF = HW // 2        # local queries per core
G = 32                # groups
N_CORES = 8
EPS = 1e-6
NG_ELEMS = (C // G) * T * HW   # elements per group in the full tensor
CB = C // 128         # 4 channel blocks
NP = CB // 2          # 2 channel-block pairs (DoubleRow k-tiles)
QG = HALF // 512      # 4 query groups of 512
NKT = HW // 128       # 32 key chunks of 128
NKP = NKT // 2        # 16 key chunk pairs
SCALE = float(C) ** -0.5
WS = 16.0             # host-side weight prescale (fp8 subnormal dodge)
SHIFT = 2.0           # exp shift: p = exp(s*SCALE - SHIFT), cancels in norm
PRE = 16.0            # attention-out prescale; == WS so bc = exactly 1/D

E4NP = ml_dtypes.float8_e4m3   # TRN fp8e4 semantics (max +-240)

BF16 = mybir.dt.bfloat16
F32 = mybir.dt.float32
FP8 = mybir.dt.float8e4
AX = mybir.AxisListType
AF = mybir.ActivationFunctionType
OP = mybir.AluOpType
DR = mybir.MatmulPerfMode.DoubleRow

_CACHE = {}


# ---------------------------------------------------------------- kernel 1
def _build_stats():
    """Partial sum/sumsq over this core's half-frame.  bf16 input halves
    the DMA; each channel-block tile is DMA'd in two halves on the two
    rings; big [128, 2048] ops amortize the per-op engine overhead (sums
    on DVE, squares+accum on ACT, running in parallel)."""
    nc = bacc.Bacc("TRN2", target_bir_lowering=False, debug=False,
                   num_devices=N_CORES)
    xh = nc.declare_dram_parameter("xh", [C, HALF], BF16, isOutput=False)
    pstats = nc.declare_dram_parameter("pstats", [128, 2 * CB], F32,
                                       isOutput=True)
    with tile.TileContext(nc) as tc:
        with tc.tile_pool(name="xt", bufs=CB) as xt_pool, \
             tc.tile_pool(name="scr", bufs=2) as scr_pool, \
             tc.tile_pool(name="st", bufs=1) as st_pool:
            stats_sb = st_pool.tile([128, 2 * CB], F32, name="stats")
            for j in range(CB):
                xt = xt_pool.tile([128, HALF], BF16, tag="xt", name="xt")
                r = xh[j * 128:(j + 1) * 128, :]
                nc.sync.dma_start(xt[:, 0:HALF // 2], r[:, 0:HALF // 2])
                nc.scalar.dma_start(xt[:, HALF // 2:HALF], r[:, HALF // 2:HALF])
                nc.vector.reduce_sum(stats_sb[:, j:j + 1], xt[:, :], axis=AX.X)
                scr = scr_pool.tile([128, HALF], F32, tag="scr", name="scr")
                nc.scalar.activation(scr[:, :], xt[:, :], AF.Square,
                                     accum_out=stats_sb[:, CB + j:CB + j + 1])
            nc.sync.dma_start(pstats[:, :], stats_sb[:, :])
    nc.finalize()
    return nc


# ---------------------------------------------------------------- kernel 2
def _body(tc, P):
    from contextlib import ExitStack

    nc = tc.nc
    with ExitStack() as ctx:
        consts = ctx.enter_context(tc.tile_pool(name="consts", bufs=1))

        # scale/bias ride the scalar HWDGE ring first (critical path for
        # the normalize), weights follow; the 8MB xf load rides sync.
        scl_sb = consts.tile([128, CB], F32, name="scl")
        nc.scalar.dma_start(scl_sb[:, :], P["scl2d"][:, :])
        bia_sb = consts.tile([128, CB], F32, name="bia")
        nc.scalar.dma_start(bia_sb[:, :], P["bia2d"][:, :])

        # local half of the frame stays resident: normalize source now,
        # residual read at proj time (saves the 4MB re-read).  DMA'd in
        # 512-token column groups below so the PE starts early.
        xloc = [consts.tile([128, HALF], F32, name=f"xloc{j}")
                for j in range(CB)]

        def wtile(nm):
            t_ = consts.tile([128, 2, 2 * C], FP8, name=nm)
            nc.scalar.dma_start(t_[:, :, :], P[nm][:, :, :])
            return t_

        wq_sb = wtile("wq8")
        wk_sb = wtile("wk8")
        wv_sb = wtile("wv8")
        wp_sb = wtile("wp8")
        bq_sb = consts.tile([128, CB], F32, name="bq")
        nc.scalar.dma_start(bq_sb[:, :], P["bq2d"][:, :])
        bpe_sb = consts.tile([128, CB], F32, name="bpe")
        nc.scalar.dma_start(bpe_sb[:, :], P["bpe2d"][:, :])

        onesf_sb = consts.tile([128, 1], F32, name="onesf")
        nc.vector.memset(onesf_sb[:, :], 1.0)
        onesrow_sb = consts.tile([1, 128], BF16, name="onesrow")
        nc.vector.memset(onesrow_sb[:, :], 1.0)
        zero_sb = consts.tile([128, 1], F32, name="zero")
        nc.vector.memset(zero_sb[:, :], 0.0)
        nsh_sb = consts.tile([128, 1], F32, name="nsh")
        nc.vector.memset(nsh_sb[:, :], -SHIFT)

        # fp8 activations, channel-block-paired for DoubleRow
        xn_pool = ctx.enter_context(tc.tile_pool(name="xn", bufs=NP))
        xn_sb = [xn_pool.tile([128, 2, HW], FP8, tag="xn", name="xn")
                 for _ in range(NP)]
        q_pool = ctx.enter_context(tc.tile_pool(name="q", bufs=NP))
        q_sb = [q_pool.tile([128, 2, HALF], FP8, tag="q", name="q")
                for _ in range(NP)]
        k_pool = ctx.enter_context(tc.tile_pool(name="k", bufs=NP))
        k_sb = [k_pool.tile([128, 2, HW], FP8, tag="k", name="k")
                for _ in range(NP)]
        v_pool = ctx.enter_context(tc.tile_pool(name="v", bufs=NKP))
        v_sb = [v_pool.tile([128, 2, C], FP8, tag="v", name="v")
                for _ in range(NKP)]

        # psum pools: 4 + 2 + 2 = 8 banks
        ps_mm = ctx.enter_context(tc.tile_pool(name="ps_mm", bufs=4, space="PSUM"))
        ps_st = ctx.enter_context(tc.tile_pool(name="ps_st", bufs=2, space="PSUM"))
        ps_pp = ctx.enter_context(tc.tile_pool(name="ps_pp", bufs=2, space="PSUM"))

        p_pool = ctx.enter_context(tc.tile_pool(name="p", bufs=4))
        acc_pool = ctx.enter_context(tc.tile_pool(name="acc", bufs=2))
        dnr_pool = ctx.enter_context(tc.tile_pool(name="dnr", bufs=2))
        bc_pool = ctx.enter_context(tc.tile_pool(name="bc", bufs=2))
        atB_pool = ctx.enter_context(tc.tile_pool(name="atB", bufs=4))
        ob_pool = ctx.enter_context(tc.tile_pool(name="ob", bufs=4))

        # ---------------- phase 0+1: streamed normalize + q/k/v -------------
        # token-group-major streaming: per 512-token group, DMA + normalize
        # its 4 channel blocks, then immediately emit every matmul that only
        # needs tokens seen so far; the PE starts ~10us earlier than with
        # half-frame-granular loads.
        def norm(j, src, cs):
            nc.vector.tensor_scalar(
                out=xn_sb[j // 2][:, j % 2, cs], in0=src,
                scalar1=scl_sb[:, j:j + 1], scalar2=bia_sb[:, j:j + 1],
                op0=OP.mult, op1=OP.add)

        def qk_group(w_sb, out_sb, j, t_, bias):
            ps = ps_mm.tile([128, 512], F32, tag="mm", name="mm")
            for ip in range(NP):
                nc.tensor.matmul(
                    ps[:, :],
                    lhsT=w_sb[:, :, ip * C + j * 128: ip * C + (j + 1) * 128],
                    rhs=xn_sb[ip][:, :, t_ * 512:(t_ + 1) * 512],
                    start=(ip == 0), stop=(ip == NP - 1), perf_mode=DR)
            dst = out_sb[j // 2][:, j % 2, t_ * 512:(t_ + 1) * 512]
            nc.scalar.activation(dst, ps[:, :], AF.Identity,
                                 scale=1.0 / WS, bias=bias)

        def v_group(m):
            ps = ps_mm.tile([128, 512], F32, tag="mm", name="mm")
            for ip in range(NP):
                nc.tensor.matmul(
                    ps[:, :],
                    lhsT=xn_sb[ip][:, :, m * 128:(m + 1) * 128],
                    rhs=wv_sb[:, :, ip * C:(ip + 1) * C],
                    start=(ip == 0), stop=(ip == NP - 1), perf_mode=DR)
            nc.vector.tensor_scalar(out=v_sb[m // 2][:, m % 2, :], in0=ps[:, :],
                                    scalar1=1.0 / WS, scalar2=None, op0=OP.mult)

        with tc.tile_pool(name="xf", bufs=8) as xf_pool:
            for tg in range(8):
                ts_, te_ = tg * 512, (tg + 1) * 512
                for j in range(CB):
                    if tg < QG:      # local half: land in the resident tiles
                        dst = xloc[j][:, ts_:te_]
                    else:
                        xt = xf_pool.tile([128, 512], F32, tag="xf", name="xf")
                        dst = xt[:, :]
                    nc.sync.dma_start(dst, P["xf"][j * 128:(j + 1) * 128,
                                                   ts_:te_])
                    norm(j, dst, slice(ts_, te_))
                if tg < QG:          # q covers exactly the local half
                    for j in range(CB):
                        qk_group(wq_sb, q_sb, j, tg, bias=bq_sb[:, j:j + 1])
                for j in range(CB):
                    qk_group(wk_sb, k_sb, j, tg, bias=zero_sb[:, :])
                for m in range(4 * tg, 4 * tg + 4):
                    v_group(m)

        # ---------------- phase 2: attention + proj per query group --------
        # proj of group g is emitted at the START of group g+1: its matmuls
        # are ready instantly (own psum bank, inputs done) and fill the PE
        # window where the next score matmuls wait on the exp lag.
        def emit_proj(atB_sb, bc, q0, pool):
            # proj matmul -> quick psum->SBUF copy (split ACT/DVE) so the
            # pp banks recycle without waiting on the bc-dependent combine;
            # the normalize+bias+residual chain then runs entirely in SBUF.
            for cb in range(CB):
                pp = pool.tile([128, 512], F32, tag="mm" if pool is ps_mm
                               else "pp", name="pp")
                for ip in range(NP):
                    nc.tensor.matmul(
                        pp[:, :],
                        lhsT=wp_sb[:, :, ip * C + cb * 128: ip * C + (cb + 1) * 128],
                        rhs=atB_sb[ip][:, :, :],
                        start=(ip == 0), stop=(ip == NP - 1), perf_mode=DR)
                ppS = ob_pool.tile([128, 512], F32, tag="t1", name="ppS")
                if cb % 2 == 0:
                    nc.scalar.copy(ppS[:, :], pp[:, :])
                else:
                    nc.vector.tensor_copy(ppS[:, :], pp[:, :])
                t1 = ob_pool.tile([128, 512], F32, tag="t1", name="t1")
                nc.vector.tensor_mul(t1[:, :], ppS[:, :], bc[:, :])
                ob = ob_pool.tile([128, 512], F32, tag="ob", name="ob")
                nc.vector.scalar_tensor_tensor(
                    ob[:, :], in0=t1[:, :], scalar=bpe_sb[:, cb:cb + 1],
                    in1=xloc[cb][:, q0:q0 + 512], op0=OP.add, op1=OP.add)
                nc.sync.dma_start(P["out"][cb * 128:(cb + 1) * 128, q0:q0 + 512],
                                  ob[:, :])

        def finalize_group(atB_sb, acc, q0, pool):
            # denominator -> bc = 1/D: ones-matmul partition-reduce of the
            # DVE partials, reciprocal (bf16: ~0.4% on 1/D, way below the
            # gate), rank-1 broadcast back to 128 partitions.  ~1.1us of
            # PE per group, with every wait already satisfied when emitted.
            dnr = ps_pp.tile([1, 512], F32, tag="pp", name="dnr")
            nc.tensor.matmul(dnr[:, :], lhsT=onesf_sb[:, :], rhs=acc[:, :],
                             start=True, stop=True)
            dnrec = dnr_pool.tile([1, 512], BF16, tag="dnr", name="dnrec")
            with nc.allow_low_precision("bf16 1/denominator on a 2e-2 gate"):
                nc.vector.reciprocal(dnrec[:, :], dnr[:, :])
            bcp = ps_pp.tile([128, 512], F32, tag="pp", name="bcp")
            nc.tensor.matmul(bcp[:, :], lhsT=onesrow_sb[:, :], rhs=dnrec[:, :],
                             start=True, stop=True)
            bc = bc_pool.tile([128, 512], F32, tag="bc", name="bc")
            nc.vector.tensor_copy(bc[:, :], bcp[:, :])
            emit_proj(atB_sb, bc, q0, pool)

        deferred = None
        for qg in range(QG):
            q0 = qg * 512
            pv = [ps_mm.tile([128, 512], F32, tag="mm", name="mm")
                  for _ in range(CB)]
            acc = acc_pool.tile([128, 512], F32, tag="acc", name="acc")

            def pvmm(m2_, p8_, start, stop):
                for cb in range(CB):
                    # attention output channel-major: out[co, qt] += v^T p
                    nc.tensor.matmul(
                        pv[cb][:, :],
                        lhsT=v_sb[m2_][:, :, cb * 128:(cb + 1) * 128],
                        rhs=p8_[:, :, :],
                        start=start, stop=stop, perf_mode=DR)

            def acc_adds(m2_, p8_):
                # denominator partials ride the DVE (the PE only ever sees
                # score/PV/proj matmuls)
                if m2_ == 0:
                    nc.vector.tensor_add(acc[:, :], p8_[:, 0, :], p8_[:, 1, :])
                else:
                    nc.vector.tensor_add(acc[:, :], acc[:, :], p8_[:, 0, :])
                    nc.vector.tensor_add(acc[:, :], acc[:, :], p8_[:, 1, :])

            # software-pipelined by two pairs: PV of pair m2-2 is emitted
            # after the scores of pair m2, so the PE has ~3us of runway at
            # a group boundary before anything depends on the previous
            # group's atB quantization or denominator chain.
            p8s = []
            for m2 in range(NKP):
                p8 = p_pool.tile([128, 2, 512], FP8, tag="p", name="p")
                for mm in range(2):
                    m = 2 * m2 + mm
                    st = ps_st.tile([128, 512], F32, tag="st", name="st")
                    for ip in range(NP):
                        nc.tensor.matmul(
                            st[:, :],
                            lhsT=k_sb[ip][:, :, m * 128:(m + 1) * 128],
                            rhs=q_sb[ip][:, :, q0:q0 + 512],
                            start=(ip == 0), stop=(ip == NP - 1), perf_mode=DR)
                    nc.scalar.activation(p8[:, mm, :], st[:, :], AF.Exp,
                                         scale=SCALE, bias=nsh_sb[:, :])
                if m2 == 1 and deferred is not None:
                    # previous group's denominator+proj: the DVE sees its
                    # reciprocal/combine chain BEFORE this group's acc adds
                    finalize_group(*deferred, pool=ps_pp)
                    deferred = None
                if m2 >= 2:
                    acc_adds(m2 - 2, p8s[m2 - 2])
                    pvmm(m2 - 2, p8s[m2 - 2], start=(m2 == 2), stop=False)
                p8s.append(p8)
            acc_adds(NKP - 2, p8s[NKP - 2])
            pvmm(NKP - 2, p8s[NKP - 2], start=False, stop=False)
            acc_adds(NKP - 1, p8s[NKP - 1])
            pvmm(NKP - 1, p8s[NKP - 1], start=False, stop=True)
            # quantize UNNORMALIZED attention out of PSUM right away (frees
            # the pv banks for the next query group); the denominator is
            # applied after the (linear) projection instead.  Split
            # ACT/DVE so the copies land in ~1.3us instead of 2.3.
            atB_sb = []
            for pr in range(NP):
                atB = atB_pool.tile([128, 2, 512], FP8, tag="atB", name="atB")
                nc.scalar.activation(atB[:, 0, :], pv[2 * pr][:, :],
                                     AF.Identity, scale=1.0 / PRE,
                                     bias=zero_sb[:, :])
                nc.vector.tensor_scalar(out=atB[:, 1, :],
                                        in0=pv[2 * pr + 1][:, :],
                                        scalar1=1.0 / PRE, scalar2=None,
                                        op0=OP.mult)
                atB_sb.append(atB)
            deferred = (atB_sb, acc, q0)
        finalize_group(*deferred, pool=ps_mm)


def _build_main():
    nc = bacc.Bacc("TRN2", target_bir_lowering=False, debug=False,
                   num_devices=N_CORES)
    P = {}
    P["xf"] = nc.declare_dram_parameter("xf", [C, HW], F32, isOutput=False)
    for nm in ("wq8", "wk8", "wv8", "wp8"):
        P[nm] = nc.declare_dram_parameter(nm, [128, 2, 2 * C], FP8,
                                          isOutput=False)
    for nm in ("bq2d", "bpe2d", "scl2d", "bia2d"):
        P[nm] = nc.declare_dram_parameter(nm, [128, CB], F32, isOutput=False)
    P["out"] = nc.declare_dram_parameter("out", [C, HALF], F32, isOutput=True)

    with tile.TileContext(nc) as tc:
        _body(tc, P)
    nc.finalize()
    return nc


def _get_ncs():
    if "nc" not in _CACHE:
        _CACHE["nc1"] = _build_stats()
        _CACHE["nc"] = _build_main()
    return _CACHE["nc1"], _CACHE["nc"]


def _frame_views(x):
    """Per-core rolled frame views: core i=(2f+h) sees frame f with its own
    half first."""
    views = []
    for i in range(N_CORES):
        f, h = divmod(i, 2)
        xfr = x[0, :, f].reshape(C, HW)
        if h == 1:
            xfr = np.concatenate([xfr[:, HALF:], xfr[:, :HALF]], axis=1)
        views.append(np.ascontiguousarray(xfr))
    return views


def _combine_stats(pstats_list, gamma, beta):
    """Host-side gather of kernel-1 partials -> per-channel scale/bias."""
    tot = np.zeros((128, 2 * CB), np.float64)
    for ps in pstats_list:
        tot += np.asarray(ps, np.float64)
    # column j holds channels [128j, 128j+128)
    s = tot[:, 0:CB].T.reshape(C)        # per-channel sum
    s2 = tot[:, CB:2 * CB].T.reshape(C)  # per-channel sumsq
    gs = s.reshape(G, C // G).sum(1)
    gs2 = s2.reshape(G, C // G).sum(1)
    meang = gs / NG_ELEMS
    varg = gs2 / NG_ELEMS - meang * meang
    rstd = 1.0 / np.sqrt(varg + EPS)
    chs = (np.asarray(gamma, np.float64) * np.repeat(rstd, C // G))
    chb = np.asarray(beta, np.float64) - np.repeat(meang, C // G) * chs
    def blk2d(v):
        return np.ascontiguousarray(v.astype(np.float32).reshape(CB, 128).T)
    return blk2d(chs), blk2d(chb)


def _w8pack(w):
    """(c_out, c_in) f32 -> [128, 2, 2C] fp8e4, channel-block-pair packed:
    out[p, i, ip*C + o] = w.T[(2*ip + i)*128 + p, o] * WS."""
    a = (np.asarray(w, np.float32).T * WS).reshape(2, 2, 128, C)
    a = np.ascontiguousarray(a.transpose(2, 1, 0, 3).reshape(128, 2, 2 * C))
    return a.astype(E4NP)


def run_with_results(inputs, trace=False, **kw):
    f32 = np.float32
    x = np.asarray(inputs["x"], f32)
    gamma = np.asarray(inputs["gamma"], f32)
    beta = np.asarray(inputs["beta"], f32)
    wq, wk, wv, wp = [np.asarray(inputs[n], f32) for n in ("wq", "wk", "wv", "wp")]
    bq, bv, bp = [np.asarray(inputs[n], f32) for n in ("bq", "bv", "bp")]

    nc1, nc2 = _get_ncs()
    views = _frame_views(x)

    # ---- launch 1: partial GroupNorm stats over disjoint half-frames
    # (bf16 input: halves the DMA; the stats shift is far below the gate)
    maps1 = [{"xh": np.ascontiguousarray(views[i][:, :HALF])
              .astype(ml_dtypes.bfloat16)}
             for i in range(N_CORES)]
    res1 = run_bass_kernel_spmd(nc1, maps1, core_ids=list(range(N_CORES)),
                                trace=trace, **kw)
    scl2d, bia2d = _combine_stats([r["pstats"] for r in res1.results],
                                  gamma, beta)

    # ---- launch 2: the block itself
    def blk2d(v):
        return np.ascontiguousarray(np.asarray(v, f32).reshape(CB, 128).T)

    shared = {
        "wq8": _w8pack(wq), "wk8": _w8pack(wk), "wv8": _w8pack(wv),
        "wp8": _w8pack(wp),
        "bq2d": blk2d(bq), "bpe2d": blk2d(bp + wp @ bv),
        "scl2d": scl2d, "bia2d": bia2d,
    }
    maps2 = [dict(shared, xf=views[i]) for i in range(N_CORES)]
    res2 = run_bass_kernel_spmd(nc2, maps2, core_ids=list(range(N_CORES)),
                                trace=trace, **kw)

    frames = []
    for f in range(T):
        a = np.asarray(res2.results[2 * f]["out"], dtype=np.float32)
        b = np.asarray(res2.results[2 * f + 1]["out"], dtype=np.float32)
        frames.append(np.concatenate([a, b], axis=1))
    out = np.stack(frames, axis=1)           # (C, T, HW)
    out = np.ascontiguousarray(out.reshape(1, C, T, 64, 64))
    return out, (res1, res2)


def kernel(**inputs):
    out, _ = run_with_results(inputs)
    return out
